# revision 8
# baseline (speedup 1.0000x reference)
"""Causal self-attention Trainium2 kernel (8 NeuronCores, SPMD).

Sharding: 8 cores = 2 batches x 4 head-groups (4 heads of 64 dims each).
Each core computes full-sequence attention for its 4 heads plus the
partial output projection for its 256 y-columns; the host sums the 4
partials per batch and adds the output bias.

v2 design: single fused pipeline tuned to keep the PE continuously busy
(the HAM throttle on TRN2 re-engages after ~5us of PE idle, halving the
matmul rate, so PE gaps cost double):
  - per j-tile rounds: attT matmul chunks -> exp on ScalarE (both heads
    per instruction) -> yT accumulation, with projection / out-proj
    matmul quanta woven between chunks as PE fillers
  - PT (exp'd attention, transposed layout) kept in SBUF in a ragged
    causal layout (only j<=i trail per j-tile), written once, read once
  - q/k projection PSUM drains on ScalarE (Copy activation, fused bias;
    Copy shares the Exp activation table so no table reloads)
  - diagonal causal masks multiplied on GpSimd (Pool), off the DVE
  - softmax denominators via the ones-column trick (row 64 of yT PSUM);
    per-ib normalization with reciprocal_approx_fast + one selector
    matmul broadcasting 1/s to all 128 partitions
  - out[t, n] partial = yTn.T @ Wp, f32, DMA'd out per 128x512 tile
"""

import sys

for _p in ("/opt/trn_rl_repo",):
    if _p not in sys.path:
        sys.path.insert(0, _p)

from contextlib import ExitStack

import ml_dtypes
import numpy as np

import concourse.bass as bass
import concourse.tile as tile
from concourse import bacc, mybir
from concourse.bass_utils import run_bass_kernel_spmd

BF16 = mybir.dt.bfloat16
F32 = mybir.dt.float32
NP_BF16 = ml_dtypes.bfloat16

B, T, C = 2, 2048, 1024
H, D = 16, 64
N_CORES = 8
GROUPS = 4          # head groups (cores per batch)
MH = C // GROUPS    # 256 columns per core (4 heads)
LH = MH // D        # 4 local heads
CT = C // 128       # 8 contraction tiles
TT = T // 128       # 16 sequence tiles of 128
IB = T // 512       # 4 i-blocks of 512
SCALE = 1.0 / np.sqrt(D)

# ragged PT layout: trail(jt) = T - 128*jt columns, cumulative offsets
TRAIL = [T - 128 * jt for jt in range(TT)]
PT_OFF = [sum(TRAIL[:jt]) for jt in range(TT)]
PT_W = sum(TRAIL)   # 17408


def _selector() -> np.ndarray:
    """sel[r, l*64+j] = 1.0 if r == l else 0, [2, 128] bf16 — K=2 matmul
    broadcasts row lh of rs2 [2, 512] to out partitions 64*lh..64*lh+64."""
    sel = np.zeros((2, 128), dtype=NP_BF16)
    sel[0, 0:64] = 1.0
    sel[1, 64:128] = 1.0
    return sel


def _tri_mask() -> np.ndarray:
    """tri[j, c] = 1.0 if j <= c else 0 (bf16), [128, 128] — multiplicative
    causal mask for the diagonal 128x128 block of each j-tile."""
    j = np.arange(128)[:, None]
    c = np.arange(128)[None, :]
    return (j <= c).astype(NP_BF16)


def emit_kernel(
    nc, xT_d, wq_d, wk_d, wv_d, wp_d, bq_d, bk_d, bv_d, out_d, tri_d, sel_d
):
    with tile.TileContext(nc) as tc, ExitStack() as ctx:
        # ---- long-lived SBUF tiles --------------------------------------
        keep = ctx.enter_context(tc.tile_pool(name="keep", bufs=1))
        xT_s = keep.tile([128, CT, T], BF16, tag="xT")
        wq_s = keep.tile([128, CT, MH], BF16, tag="wq")
        wk_s = keep.tile([128, CT, MH], BF16, tag="wk")
        wv_s = keep.tile([128, CT, MH], BF16, tag="wv")
        wp_s = keep.tile([128, 2, C], BF16, tag="wp")
        qT_s = keep.tile([128, 2, T], BF16, tag="qT")
        kT_s = keep.tile([128, 2, T], BF16, tag="kT")
        v_s = keep.tile([128, TT, LH, D + 1], BF16, tag="v")
        PT_s = keep.tile([128, 2, PT_W], BF16, tag="PT")
        yTn_s = keep.tile([128, 2, T], BF16, tag="yTn")
        tri_st = keep.tile([128, 128], BF16, tag="tri_st")
        tri_s = keep.tile([128, 128], BF16, tag="tri")
        sel_st = keep.tile([2, 128], BF16, tag="sel_st")
        sel_s = keep.tile([2, 128], BF16, tag="sel")
        bq_st = keep.tile([128, 2], F32, tag="bq_st")
        bq_s = keep.tile([128, 2], F32, tag="bq")
        bk_st = keep.tile([128, 2], F32, tag="bk_st")
        bk_s = keep.tile([128, 2], F32, tag="bk")
        bv_row = keep.tile([1, MH], F32, tag="bv_row")
        bv_row_bf = keep.tile([1, MH], BF16, tag="bv_row_bf")
        bv_bc = keep.tile([128, MH], F32, tag="bv_bc")
        ones_bf128 = keep.tile([1, 128], BF16, tag="ones_bf128")

        # ---- pools ------------------------------------------------------
        att = ctx.enter_context(
            tc.tile_pool(name="att", bufs=2, space="PSUM"))      # 2x2 banks
        yps = ctx.enter_context(
            tc.tile_pool(name="yps", bufs=1, space="PSUM"))      # 2x1 bank
        mmp = ctx.enter_context(
            tc.tile_pool(name="mmp", bufs=1, space="PSUM"))      # 1 bank
        sps = ctx.enter_context(
            tc.tile_pool(name="sps", bufs=1, space="PSUM"))      # 1 bank
        norm = ctx.enter_context(tc.tile_pool(name="norm", bufs=2))
        osb = ctx.enter_context(tc.tile_pool(name="osb", bufs=2))

        # ---- input DMAs (SP queue, first-needed first) ------------------
        xT_r = xT_d.ap().rearrange("(o p) t -> p o t", p=128)
        wq_r = wq_d.ap().rearrange("(o p) m -> p o m", p=128)
        wk_r = wk_d.ap().rearrange("(o p) m -> p o m", p=128)
        wv_r = wv_d.ap().rearrange("(o p) m -> p o m", p=128)
        wp_r = wp_d.ap().rearrange("(o p) n -> p o n", p=128)

        def xt_chunk(tb):
            nc.sync.dma_start(
                xT_s[:, :, tb * 512 : (tb + 1) * 512],
                xT_r[:, :, tb * 512 : (tb + 1) * 512],
            )

        nc.sync.dma_start(wq_s[:], wq_r[:])
        nc.sync.dma_start(wk_s[:], wk_r[:])
        xt_chunk(0)
        xt_chunk(1)
        xt_chunk(2)
        xt_chunk(3)
        nc.sync.dma_start(wv_s[:], wv_r[:])
        nc.sync.dma_start(wp_s[:], wp_r[:])
        # consts staged through a DVE copy: consumers then depend on DVE
        # program order instead of a DMA semaphore (walrus 1-wait limit)
        nc.gpsimd.dma_start(tri_st[:], tri_d.ap())
        nc.gpsimd.dma_start(sel_st[:], sel_d.ap())
        nc.gpsimd.dma_start(bq_st[:], bq_d.ap().rearrange("(o p) -> p o", p=128))
        nc.gpsimd.dma_start(bk_st[:], bk_d.ap().rearrange("(o p) -> p o", p=128))
        nc.gpsimd.dma_start(bv_row[:], bv_d.ap()[None, :])
        nc.vector.tensor_copy(tri_s[:], tri_st[:])
        nc.vector.tensor_copy(sel_s[:], sel_st[:])
        nc.vector.tensor_copy(bq_s[:], bq_st[:])
        nc.vector.tensor_copy(bk_s[:], bk_st[:])
        nc.vector.tensor_copy(bv_row_bf[:], bv_row[:])
        nc.vector.memset(ones_bf128[:], 1.0)
        nc.vector.memset(v_s[:, :, :, D : D + 1], 1.0)
        bv_ps = mmp.tile([128, 512], F32, tag="mm", name="bv_ps")
        nc.tensor.matmul(
            bv_ps[:, :MH], ones_bf128[:], bv_row_bf[:], start=True, stop=True
        )
        nc.vector.tensor_copy(bv_bc[:], bv_ps[:, :MH])

        # ---- emit helpers ----------------------------------------------
        def qk_block(w_s, b_s, dst, mt, tb, pool, pool_kw):
            """q/k projection for one (mt, tb): 8 matmuls + DVE drain (the
            ScalarE is the pacing engine during attention, keep it clear)."""
            ps = pool.tile([128, 512], F32, **pool_kw)
            for ct in range(CT):
                nc.tensor.matmul(
                    ps[:],
                    w_s[:, ct, mt * 128 : (mt + 1) * 128],
                    xT_s[:, ct, tb * 512 : (tb + 1) * 512],
                    start=(ct == 0),
                    stop=(ct == CT - 1),
                )
            nc.vector.tensor_scalar(
                dst[:, mt, tb * 512 : (tb + 1) * 512],
                ps[:],
                b_s[:, mt : mt + 1],
                None,
                mybir.AluOpType.add,
            )

        def v_block(tt):
            """v projection for one tt: 8 matmuls + DVE drain (bias add)."""
            ps = mmp.tile([128, 512], F32, tag="mm", name="v_ps")
            for ct in range(CT):
                nc.tensor.matmul(
                    ps[:, :MH],
                    xT_s[:, ct, tt * 128 : (tt + 1) * 128],
                    wv_s[:, ct, :],
                    start=(ct == 0),
                    stop=(ct == CT - 1),
                )
            nc.vector.tensor_tensor(
                v_s[:, tt, :, 0:D],
                ps[:, :MH].rearrange("p (h d) -> p h d", h=LH),
                bv_bc[:].rearrange("p (h d) -> p h d", h=LH),
                mybir.AluOpType.add,
            )

        in_tail = [False]
        ostate = {"n": 0, "ot": None}

        def out_block(tt, nb):
            """out projection for one (tt, nb): 2 matmuls + drain; full
            [128, 1024] DMA per tt (coalesced across nb). PSUM rotates
            mm/sps (att pool in tail); drains alternate DVE/ScalarE;
            DMA queues alternate sync/gpsimd."""
            k = ostate["n"]
            ostate["n"] += 1
            if in_tail[0]:
                ab = att.tile([128, 2, 512], F32, tag="att", name="o_ps")
                ps = ab[:, 0, :]
            elif k % 2 == 0:
                ps = mmp.tile([128, 512], F32, tag="mm", name="o_ps")
            else:
                ps = sps.tile([128, 512], F32, tag="S", name="o_ps")
            for pp in range(2):
                nc.tensor.matmul(
                    ps,
                    yTn_s[:, pp, tt * 128 : (tt + 1) * 128],
                    wp_s[:, pp, nb * 512 : (nb + 1) * 512],
                    start=(pp == 0),
                    stop=(pp == 1),
                )
            if nb == 0:
                ostate["ot"] = osb.tile(
                    [128, 1024], BF16, tag="out_t", name="out_t"
                )
            ot = ostate["ot"]
            if in_tail[0] or k % 2 == 1:
                nc.scalar.activation(
                    ot[:, nb * 512 : (nb + 1) * 512],
                    ps,
                    mybir.ActivationFunctionType.Copy,
                )
            else:
                nc.vector.tensor_copy(ot[:, nb * 512 : (nb + 1) * 512], ps)
            if nb == 1:
                eng = nc.sync if (tt % 2 == 0) else nc.gpsimd
                eng.dma_start(out_r[tt, :, :], ot[:])

        out_r = out_d.ap().rearrange("(tt p) n -> tt p n", p=128)

        # filler queue: list of zero-arg closures, each one PE quantum
        fillers = []

        def pump(n=1):
            for _ in range(n):
                if fillers:
                    fillers.pop(0)()

        # ---- phase A: q/k projections for pair 0 (att pool as psum) ----
        for tb in range(IB):
            ab = att.tile([128, 2, 512], F32, tag="att", name="ab")
            # use the two halves of an att tile for q and k drains
            for half, (w_s, b_s, dst) in enumerate(
                ((wq_s, bq_s, qT_s), (wk_s, bk_s, kT_s))
            ):
                for ct in range(CT):
                    nc.tensor.matmul(
                        ab[:, half, :],
                        w_s[:, ct, 0:128],
                        xT_s[:, ct, tb * 512 : (tb + 1) * 512],
                        start=(ct == 0),
                        stop=(ct == CT - 1),
                    )
                nc.scalar.activation(
                    dst[:, 0, tb * 512 : (tb + 1) * 512],
                    ab[:, half, :],
                    mybir.ActivationFunctionType.Identity,
                    bias=b_s[:, 0:1],
                )

        # fillers for pair-0 attention phase: v blocks + q/k mt=1 blocks
        for tt in range(TT):
            fillers.append(lambda tt=tt: v_block(tt))
            if tt % 2 == 1:
                tb = tt // 2
                if tb < IB:
                    fillers.append(
                        lambda tb=tb: qk_block(
                            wq_s, bq_s, qT_s, 1, tb, mmp, dict(tag="mm", name="q_ps")
                        )
                    )
                    fillers.append(
                        lambda tb=tb: qk_block(
                            wk_s, bk_s, kT_s, 1, tb, mmp, dict(tag="mm", name="k_ps")
                        )
                    )

        # ---- attention pairs --------------------------------------------
        def attT_trail(p, jt):
            """attT + exp + diag mask for j-tile jt, full causal trail."""
            ia = 128 * jt
            trail = TRAIL[jt]
            off = PT_OFF[jt]
            c = 0
            while c < trail:
                n = min(512, trail - c)
                ab = att.tile([128, 2, 512], F32, tag="att", name="ab")
                for lh in range(2):
                    prow = slice(64 * lh, 64 * lh + 64)
                    nc.tensor.matmul(
                        ab[:, lh, :n],
                        kT_s[prow, p, ia : ia + 128],
                        qT_s[prow, p, ia + c : ia + c + n],
                        start=True,
                        stop=True,
                    )
                pump(1)
                nc.scalar.activation(
                    PT_s[:, :, off + c : off + c + n],
                    ab[:, :, :n],
                    mybir.ActivationFunctionType.Exp,
                    scale=float(SCALE),
                )
                c += n
            # diagonal causal mask on GpSimd (Pool), off the DVE
            for lh in range(2):
                nc.gpsimd.tensor_tensor(
                    PT_s[:, lh, off : off + 128],
                    PT_s[:, lh, off : off + 128],
                    tri_s[:],
                    mybir.AluOpType.mult,
                )

        for p in range(2):
            for ib in range(IB):
                # yT accumulation interleaved with the 4 new j-tiles' attT
                # trails (emitted one j-tile ahead so exp + mask are done
                # before the consuming yT matmul; ScalarE stays fed during
                # the yT burst instead of starving then refilling cold)
                yts = [
                    yps.tile([D + 1, 512], F32, tag=f"y{lh}", name=f"yt{lh}")
                    for lh in range(2)
                ]
                attT_trail(p, 4 * ib)
                for jt2 in range(4 * ib + 4):
                    if 4 * ib <= jt2 <= 4 * ib + 2:
                        attT_trail(p, jt2 + 1)
                    ia2 = 128 * jt2
                    c0 = max(512 * ib, ia2)
                    w = 512 * ib + 512 - c0
                    for lh in range(2):
                        nc.tensor.matmul(
                            yts[lh][:, c0 - 512 * ib : 512],
                            v_s[:, jt2, 2 * p + lh, :],
                            PT_s[:, lh, PT_OFF[jt2] + c0 - ia2 :
                                 PT_OFF[jt2] + c0 - ia2 + w],
                            start=(jt2 == 0),
                            stop=(jt2 == 4 * ib + 3),
                        )
                    if jt2 % 2 == 0:
                        pump(1)

                # drain + normalize this ib
                srows = norm.tile([2, 512], F32, tag="srows", name="srows")
                rs2f = norm.tile([2, 512], F32, tag="rs2f", name="rs2f")
                rs2 = norm.tile([2, 512], BF16, tag="rs2", name="rs2")
                yTall = [
                    norm.tile([D + 1, 512], F32, tag=f"yTall{lh}", name=f"yTall{lh}")
                    for lh in range(2)
                ]
                for lh in range(2):
                    nc.vector.tensor_copy(yTall[lh][:], yts[lh][:])
                    nc.sync.dma_start(srows[lh : lh + 1, :], yTall[lh][D : D + 1, :])
                nc.vector.reciprocal_approx_fast(rs2f[:], srows[:])
                nc.vector.tensor_copy(rs2[:], rs2f[:])
                pump(1)
                S_ps = sps.tile([128, 512], F32, tag="S", name="S_ps")
                nc.tensor.matmul(S_ps[:], sel_s[:], rs2[:], start=True, stop=True)
                for lh in range(2):
                    nc.vector.tensor_tensor(
                        yTn_s[64 * lh : 64 * lh + 64, p, 512 * ib : 512 * ib + 512],
                        yTall[lh][0:D, :],
                        S_ps[64 * lh : 64 * lh + 64, :],
                        mybir.AluOpType.mult,
                    )
                if p == 1:
                    # both pairs normalized for this ib: queue out-proj fillers
                    for tt in range(4 * ib, 4 * ib + 4):
                        for nb in range(2):
                            fillers.append(
                                lambda tt=tt, nb=nb: out_block(tt, nb)
                            )

        # tail: drain remaining fillers (last ib's out-proj) on att pool
        in_tail[0] = True
        while fillers:
            f = fillers.pop(0)
            f()


_NC_CACHE = None


def get_nc() -> bass.Bass:
    global _NC_CACHE
    if _NC_CACHE is None:
        nc = bacc.Bacc()
        xT_d = nc.declare_dram_parameter("xT", [C, T], BF16, isOutput=False)
        wq_d = nc.declare_dram_parameter("wq", [C, MH], BF16, isOutput=False)
        wk_d = nc.declare_dram_parameter("wk", [C, MH], BF16, isOutput=False)
        wv_d = nc.declare_dram_parameter("wv", [C, MH], BF16, isOutput=False)
        wp_d = nc.declare_dram_parameter("wp", [MH, C], BF16, isOutput=False)
        bq_d = nc.declare_dram_parameter("bq", [MH], F32, isOutput=False)
        bk_d = nc.declare_dram_parameter("bk", [MH], F32, isOutput=False)
        bv_d = nc.declare_dram_parameter("bv", [MH], F32, isOutput=False)
        out_d = nc.declare_dram_parameter("out", [T, C], BF16, isOutput=True)
        tri_d = nc.inline_tensor(_tri_mask(), name="tri_mask")
        sel_d = nc.inline_tensor(_selector(), name="selector")
        emit_kernel(
            nc, xT_d, wq_d, wk_d, wv_d, wp_d, bq_d, bk_d, bv_d, out_d, tri_d, sel_d
        )
        nc.finalize()
        _NC_CACHE = nc
    return _NC_CACHE


def make_in_maps(x, Wq, bq, Wk, bk, Wv, bv, Wp, bp):
    in_maps = []
    for core in range(N_CORES):
        b, g = divmod(core, GROUPS)
        sl = slice(g * MH, (g + 1) * MH)
        in_maps.append(
            {
                "xT": np.ascontiguousarray(x[b].T).astype(NP_BF16),
                "wq": np.ascontiguousarray(Wq[:, sl]).astype(NP_BF16),
                "wk": np.ascontiguousarray(Wk[:, sl]).astype(NP_BF16),
                "wv": np.ascontiguousarray(Wv[:, sl]).astype(NP_BF16),
                "wp": np.ascontiguousarray(Wp[sl, :]).astype(NP_BF16),
                "bq": np.ascontiguousarray(bq[sl]).astype(np.float32),
                "bk": np.ascontiguousarray(bk[sl]).astype(np.float32),
                "bv": np.ascontiguousarray(bv[sl]).astype(np.float32),
            }
        )
    return in_maps


def kernel(x, Wq, bq, Wk, bk, Wv, bv, Wp, bp, _results_hook=None, _trace=False):
    x = np.asarray(x, dtype=np.float32)
    nc = get_nc()
    in_maps = make_in_maps(x, Wq, bq, Wk, bk, Wv, bv, Wp, bp)
    res = run_bass_kernel_spmd(
        nc, in_maps, core_ids=list(range(N_CORES)), trace=_trace
    )
    if _results_hook is not None:
        _results_hook(res)
    out = np.zeros((B, T, C), dtype=np.float32)
    for core in range(N_CORES):
        b = core // GROUPS
        out[b] += res.results[core]["out"].astype(np.float32)
    out += np.asarray(bp, dtype=np.float32)[None, None, :]
    return out


# revision 9
# speedup vs baseline: 1.1323x; 1.1323x over previous
"""Causal self-attention Trainium2 kernel (8 NeuronCores, SPMD).

Sharding: 8 cores = 2 batches x 4 head-groups (4 heads of 64 dims each).
Each core computes full-sequence attention for its 4 heads plus the
partial output projection for its 256 y-columns; the host sums the 4
partials per batch and adds the output bias.

v2 design: single fused pipeline tuned to keep the PE continuously busy
(the HAM throttle on TRN2 re-engages after ~5us of PE idle, halving the
matmul rate, so PE gaps cost double):
  - per j-tile rounds: attT matmul chunks -> exp on ScalarE (both heads
    per instruction) -> yT accumulation, with projection / out-proj
    matmul quanta woven between chunks as PE fillers
  - PT (exp'd attention, transposed layout) kept in SBUF in a ragged
    causal layout (only j<=i trail per j-tile), written once, read once
  - q/k projection PSUM drains on ScalarE (Copy activation, fused bias;
    Copy shares the Exp activation table so no table reloads)
  - diagonal causal masks multiplied on GpSimd (Pool), off the DVE
  - softmax denominators via the ones-column trick (row 64 of yT PSUM);
    per-ib normalization with reciprocal_approx_fast + one selector
    matmul broadcasting 1/s to all 128 partitions
  - out[t, n] partial = yTn.T @ Wp, f32, DMA'd out per 128x512 tile
"""

import sys

for _p in ("/opt/trn_rl_repo",):
    if _p not in sys.path:
        sys.path.insert(0, _p)

from contextlib import ExitStack

import ml_dtypes
import numpy as np

import concourse.bass as bass
import concourse.tile as tile
from concourse import bacc, mybir
from concourse.bass_utils import run_bass_kernel_spmd

BF16 = mybir.dt.bfloat16
F32 = mybir.dt.float32
NP_BF16 = ml_dtypes.bfloat16

B, T, C = 2, 2048, 1024
H, D = 16, 64
N_CORES = 8
GROUPS = 4          # head groups (cores per batch)
MH = C // GROUPS    # 256 columns per core (4 heads)
LH = MH // D        # 4 local heads
CT = C // 128       # 8 contraction tiles
TT = T // 128       # 16 sequence tiles of 128
IB = T // 512       # 4 i-blocks of 512
SCALE = 1.0 / np.sqrt(D)

# ragged PT layout: trail(jt) = T - 128*jt columns, cumulative offsets
TRAIL = [T - 128 * jt for jt in range(TT)]
PT_OFF = [sum(TRAIL[:jt]) for jt in range(TT)]
PT_W = sum(TRAIL)   # 17408


def _selector() -> np.ndarray:
    """sel[r, l*64+j] = 1.0 if r == l else 0, [2, 128] bf16 — K=2 matmul
    broadcasts row lh of rs2 [2, 512] to out partitions 64*lh..64*lh+64."""
    sel = np.zeros((2, 128), dtype=NP_BF16)
    sel[0, 0:64] = 1.0
    sel[1, 64:128] = 1.0
    return sel


def _tri_mask() -> np.ndarray:
    """tri[j, c] = 1.0 if j <= c else 0 (bf16), [128, 128] — multiplicative
    causal mask for the diagonal 128x128 block of each j-tile."""
    j = np.arange(128)[:, None]
    c = np.arange(128)[None, :]
    return (j <= c).astype(NP_BF16)


def emit_kernel(
    nc, xT_d, wq_d, wk_d, wv_d, wp_d, bq_d, bk_d, bv_d, out_d, tri_d, sel_d
):
    with tile.TileContext(nc) as tc, ExitStack() as ctx:
        # ---- long-lived SBUF tiles --------------------------------------
        keep = ctx.enter_context(tc.tile_pool(name="keep", bufs=1))
        xT_s = keep.tile([128, CT, T], BF16, tag="xT")
        wq_s = keep.tile([128, CT, MH], BF16, tag="wq")
        wk_s = keep.tile([128, CT, MH], BF16, tag="wk")
        wv_s = keep.tile([128, CT, MH], BF16, tag="wv")
        wp_s = keep.tile([128, 2, C], BF16, tag="wp")
        qT_s = keep.tile([128, 2, T], BF16, tag="qT")
        kT_s = keep.tile([128, 2, T], BF16, tag="kT")
        v_s = keep.tile([128, TT, LH, D + 1], BF16, tag="v")
        PT_s = keep.tile([128, 2, PT_W], BF16, tag="PT")
        yTn_s = keep.tile([128, 2, T], BF16, tag="yTn")
        tri_st = keep.tile([128, 128], BF16, tag="tri_st")
        tri_s = keep.tile([128, 128], BF16, tag="tri")
        sel_st = keep.tile([2, 128], BF16, tag="sel_st")
        sel_s = keep.tile([2, 128], BF16, tag="sel")
        bq_st = keep.tile([128, 2], F32, tag="bq_st")
        bq_s = keep.tile([128, 2], F32, tag="bq")
        bk_st = keep.tile([128, 2], F32, tag="bk_st")
        bk_s = keep.tile([128, 2], F32, tag="bk")
        bv_row = keep.tile([1, MH], F32, tag="bv_row")
        bv_row_bf = keep.tile([1, MH], BF16, tag="bv_row_bf")
        bv_bc = keep.tile([128, MH], F32, tag="bv_bc")
        ones_bf128 = keep.tile([1, 128], BF16, tag="ones_bf128")

        # ---- pools ------------------------------------------------------
        att = ctx.enter_context(
            tc.tile_pool(name="att", bufs=2, space="PSUM"))      # 2x2 banks
        yps = ctx.enter_context(
            tc.tile_pool(name="yps", bufs=1, space="PSUM"))      # 2x1 bank
        mmp = ctx.enter_context(
            tc.tile_pool(name="mmp", bufs=2, space="PSUM"))      # 2 banks
        norm = ctx.enter_context(tc.tile_pool(name="norm", bufs=2))
        osb = ctx.enter_context(tc.tile_pool(name="osb", bufs=2))

        # ---- input DMAs (SP queue, first-needed first) ------------------
        xT_r = xT_d.ap().rearrange("(o p) t -> p o t", p=128)
        wq_r = wq_d.ap().rearrange("(o p) m -> p o m", p=128)
        wk_r = wk_d.ap().rearrange("(o p) m -> p o m", p=128)
        wv_r = wv_d.ap().rearrange("(o p) m -> p o m", p=128)
        wp_r = wp_d.ap().rearrange("(o p) n -> p o n", p=128)

        def xt_chunk(tb):
            nc.sync.dma_start(
                xT_s[:, :, tb * 512 : (tb + 1) * 512],
                xT_r[:, :, tb * 512 : (tb + 1) * 512],
            )

        nc.sync.dma_start(wq_s[:], wq_r[:])
        nc.sync.dma_start(wk_s[:], wk_r[:])
        xt_chunk(0)
        xt_chunk(1)
        xt_chunk(2)
        xt_chunk(3)
        nc.sync.dma_start(wv_s[:], wv_r[:])
        nc.sync.dma_start(wp_s[:], wp_r[:])
        # consts staged through a DVE copy: consumers then depend on DVE
        # program order instead of a DMA semaphore (walrus 1-wait limit)
        nc.gpsimd.dma_start(tri_st[:], tri_d.ap())
        nc.gpsimd.dma_start(sel_st[:], sel_d.ap())
        nc.gpsimd.dma_start(bq_st[:], bq_d.ap().rearrange("(o p) -> p o", p=128))
        nc.gpsimd.dma_start(bk_st[:], bk_d.ap().rearrange("(o p) -> p o", p=128))
        nc.gpsimd.dma_start(bv_row[:], bv_d.ap()[None, :])
        nc.vector.tensor_copy(tri_s[:], tri_st[:])
        nc.vector.tensor_copy(sel_s[:], sel_st[:])
        nc.vector.tensor_copy(bq_s[:], bq_st[:])
        nc.vector.tensor_copy(bk_s[:], bk_st[:])
        nc.vector.tensor_copy(bv_row_bf[:], bv_row[:])
        nc.vector.memset(ones_bf128[:], 1.0)
        nc.vector.memset(v_s[:, :, :, D : D + 1], 1.0)
        bv_ps = mmp.tile([128, 512], F32, tag="mm", name="bv_ps")
        nc.tensor.matmul(
            bv_ps[:, :MH], ones_bf128[:], bv_row_bf[:], start=True, stop=True
        )
        nc.vector.tensor_copy(bv_bc[:], bv_ps[:, :MH])

        # ---- emit helpers ----------------------------------------------
        def qk_block(w_s, b_s, dst, mt, tb, pool, pool_kw):
            """q/k projection for one (mt, tb): 8 matmuls + DVE drain (the
            ScalarE is the pacing engine during attention, keep it clear)."""
            ps = pool.tile([128, 512], F32, **pool_kw)
            for ct in range(CT):
                nc.tensor.matmul(
                    ps[:],
                    w_s[:, ct, mt * 128 : (mt + 1) * 128],
                    xT_s[:, ct, tb * 512 : (tb + 1) * 512],
                    start=(ct == 0),
                    stop=(ct == CT - 1),
                )
            nc.vector.tensor_scalar(
                dst[:, mt, tb * 512 : (tb + 1) * 512],
                ps[:],
                b_s[:, mt : mt + 1],
                None,
                mybir.AluOpType.add,
            )

        def v_block(tt):
            """v projection for one tt: 8 matmuls + DVE drain (bias add)."""
            ps = mmp.tile([128, 512], F32, tag="mm", name="v_ps")
            for ct in range(CT):
                nc.tensor.matmul(
                    ps[:, :MH],
                    xT_s[:, ct, tt * 128 : (tt + 1) * 128],
                    wv_s[:, ct, :],
                    start=(ct == 0),
                    stop=(ct == CT - 1),
                )
            nc.vector.tensor_tensor(
                v_s[:, tt, :, 0:D],
                ps[:, :MH].rearrange("p (h d) -> p h d", h=LH),
                bv_bc[:].rearrange("p (h d) -> p h d", h=LH),
                mybir.AluOpType.add,
            )

        in_tail = [False]
        ostate = {"n": 0, "ot": None}

        def out_block(tt, nb):
            """out projection for one (tt, nb): 2 matmuls + drain; full
            [128, 1024] DMA per tt (coalesced across nb). PSUM rotates
            mm/sps (att pool in tail); drains alternate DVE/ScalarE;
            DMA queues alternate sync/gpsimd."""
            k = ostate["n"]
            ostate["n"] += 1
            if in_tail[0]:
                ab = att.tile([128, 2, 512], F32, tag="att", name="o_ps")
                ps = ab[:, 0, :]
            else:
                ps = mmp.tile([128, 512], F32, tag="mm", name="o_ps")
            for pp in range(2):
                nc.tensor.matmul(
                    ps,
                    yTn_s[:, pp, tt * 128 : (tt + 1) * 128],
                    wp_s[:, pp, nb * 512 : (nb + 1) * 512],
                    start=(pp == 0),
                    stop=(pp == 1),
                )
            if nb == 0:
                ostate["ot"] = osb.tile(
                    [128, 1024], BF16, tag="out_t", name="out_t"
                )
            ot = ostate["ot"]
            if in_tail[0]:
                nc.scalar.activation(
                    ot[:, nb * 512 : (nb + 1) * 512],
                    ps,
                    mybir.ActivationFunctionType.Copy,
                )
            else:
                nc.vector.tensor_copy(ot[:, nb * 512 : (nb + 1) * 512], ps)
            if nb == 1:
                eng = nc.sync if (tt % 2 == 0) else nc.gpsimd
                eng.dma_start(out_r[tt, :, :], ot[:])

        out_r = out_d.ap().rearrange("(tt p) n -> tt p n", p=128)

        # filler queue: list of zero-arg closures, each one PE quantum
        fillers = []

        def pump(n=1):
            for _ in range(n):
                if fillers:
                    fillers.pop(0)()

        # ---- phase A: q/k projections for pair 0 (att pool as psum) ----
        for tb in range(IB):
            ab = att.tile([128, 2, 512], F32, tag="att", name="ab")
            # use the two halves of an att tile for q and k drains
            for half, (w_s, b_s, dst) in enumerate(
                ((wq_s, bq_s, qT_s), (wk_s, bk_s, kT_s))
            ):
                for ct in range(CT):
                    nc.tensor.matmul(
                        ab[:, half, :],
                        w_s[:, ct, 0:128],
                        xT_s[:, ct, tb * 512 : (tb + 1) * 512],
                        start=(ct == 0),
                        stop=(ct == CT - 1),
                    )
                nc.scalar.activation(
                    dst[:, 0, tb * 512 : (tb + 1) * 512],
                    ab[:, half, :],
                    mybir.ActivationFunctionType.Identity,
                    bias=b_s[:, 0:1],
                )

        # fillers for pair-0 attention phase: v blocks + q/k mt=1 blocks
        for tt in range(TT):
            fillers.append(lambda tt=tt: v_block(tt))
            if tt % 2 == 1:
                tb = tt // 2
                if tb < IB:
                    fillers.append(
                        lambda tb=tb: qk_block(
                            wq_s, bq_s, qT_s, 1, tb, mmp, dict(tag="mm", name="q_ps")
                        )
                    )
                    fillers.append(
                        lambda tb=tb: qk_block(
                            wk_s, bk_s, kT_s, 1, tb, mmp, dict(tag="mm", name="k_ps")
                        )
                    )

        # ---- attention pairs --------------------------------------------
        def attT_trail(p, jt):
            """attT + exp + diag mask for j-tile jt, full causal trail."""
            ia = 128 * jt
            trail = TRAIL[jt]
            off = PT_OFF[jt]
            c = 0
            while c < trail:
                n = min(512, trail - c)
                ab = att.tile([128, 2, 512], F32, tag="att", name="ab")
                for lh in range(2):
                    prow = slice(64 * lh, 64 * lh + 64)
                    nc.tensor.matmul(
                        ab[:, lh, :n],
                        kT_s[prow, p, ia : ia + 128],
                        qT_s[prow, p, ia + c : ia + c + n],
                        start=True,
                        stop=True,
                    )
                pump(1)
                nc.scalar.activation(
                    PT_s[:, :, off + c : off + c + n],
                    ab[:, :, :n],
                    mybir.ActivationFunctionType.Exp,
                    scale=float(SCALE),
                )
                c += n
            # diagonal causal mask on GpSimd (Pool), off the DVE
            for lh in range(2):
                nc.gpsimd.tensor_tensor(
                    PT_s[:, lh, off : off + 128],
                    PT_s[:, lh, off : off + 128],
                    tri_s[:],
                    mybir.AluOpType.mult,
                )

        for p in range(2):
            for ib in range(IB):
                # yT accumulation interleaved with the 4 new j-tiles' attT
                # trails (emitted one j-tile ahead so exp + mask are done
                # before the consuming yT matmul; ScalarE stays fed during
                # the yT burst instead of starving then refilling cold)
                yts = [
                    yps.tile([D + 1, 512], F32, tag=f"y{lh}", name=f"yt{lh}")
                    for lh in range(2)
                ]
                attT_trail(p, 4 * ib)
                for jt2 in range(4 * ib + 4):
                    if 4 * ib <= jt2 <= 4 * ib + 2:
                        attT_trail(p, jt2 + 1)
                    ia2 = 128 * jt2
                    c0 = max(512 * ib, ia2)
                    w = 512 * ib + 512 - c0
                    for lh in range(2):
                        nc.tensor.matmul(
                            yts[lh][:, c0 - 512 * ib : 512],
                            v_s[:, jt2, 2 * p + lh, :],
                            PT_s[:, lh, PT_OFF[jt2] + c0 - ia2 :
                                 PT_OFF[jt2] + c0 - ia2 + w],
                            start=(jt2 == 0),
                            stop=(jt2 == 4 * ib + 3),
                        )
                    if jt2 % 2 == 0:
                        pump(1)

                # drain + normalize this ib
                srows = norm.tile([2, 512], F32, tag="srows", name="srows")
                rs2f = norm.tile([2, 512], F32, tag="rs2f", name="rs2f")
                rs2 = norm.tile([2, 512], BF16, tag="rs2", name="rs2")
                yTall = [
                    norm.tile([D + 1, 512], F32, tag=f"yTall{lh}", name=f"yTall{lh}")
                    for lh in range(2)
                ]
                for lh in range(2):
                    nc.vector.tensor_copy(yTall[lh][:], yts[lh][:])
                    nc.sync.dma_start(srows[lh : lh + 1, :], yTall[lh][D : D + 1, :])
                nc.vector.reciprocal_approx_fast(rs2f[:], srows[:])
                nc.vector.tensor_copy(rs2[:], rs2f[:])
                pump(1)
                S_ps = yps.tile([128, 512], F32, tag="y0", name="S_ps")
                nc.tensor.matmul(S_ps[:], sel_s[:], rs2[:], start=True, stop=True)
                for lh in range(2):
                    nc.vector.tensor_tensor(
                        yTn_s[64 * lh : 64 * lh + 64, p, 512 * ib : 512 * ib + 512],
                        yTall[lh][0:D, :],
                        S_ps[64 * lh : 64 * lh + 64, :],
                        mybir.AluOpType.mult,
                    )
                if p == 1:
                    # both pairs normalized for this ib: queue out-proj fillers
                    for tt in range(4 * ib, 4 * ib + 4):
                        for nb in range(2):
                            fillers.append(
                                lambda tt=tt, nb=nb: out_block(tt, nb)
                            )

        # tail: drain remaining fillers (last ib's out-proj) on att pool
        in_tail[0] = True
        while fillers:
            f = fillers.pop(0)
            f()


_NC_CACHE = None


def get_nc() -> bass.Bass:
    global _NC_CACHE
    if _NC_CACHE is None:
        nc = bacc.Bacc()
        xT_d = nc.declare_dram_parameter("xT", [C, T], BF16, isOutput=False)
        wq_d = nc.declare_dram_parameter("wq", [C, MH], BF16, isOutput=False)
        wk_d = nc.declare_dram_parameter("wk", [C, MH], BF16, isOutput=False)
        wv_d = nc.declare_dram_parameter("wv", [C, MH], BF16, isOutput=False)
        wp_d = nc.declare_dram_parameter("wp", [MH, C], BF16, isOutput=False)
        bq_d = nc.declare_dram_parameter("bq", [MH], F32, isOutput=False)
        bk_d = nc.declare_dram_parameter("bk", [MH], F32, isOutput=False)
        bv_d = nc.declare_dram_parameter("bv", [MH], F32, isOutput=False)
        out_d = nc.declare_dram_parameter("out", [T, C], BF16, isOutput=True)
        tri_d = nc.inline_tensor(_tri_mask(), name="tri_mask")
        sel_d = nc.inline_tensor(_selector(), name="selector")
        emit_kernel(
            nc, xT_d, wq_d, wk_d, wv_d, wp_d, bq_d, bk_d, bv_d, out_d, tri_d, sel_d
        )
        nc.finalize()
        _NC_CACHE = nc
    return _NC_CACHE


def make_in_maps(x, Wq, bq, Wk, bk, Wv, bv, Wp, bp):
    in_maps = []
    for core in range(N_CORES):
        b, g = divmod(core, GROUPS)
        sl = slice(g * MH, (g + 1) * MH)
        in_maps.append(
            {
                "xT": np.ascontiguousarray(x[b].T).astype(NP_BF16),
                "wq": np.ascontiguousarray(Wq[:, sl]).astype(NP_BF16),
                "wk": np.ascontiguousarray(Wk[:, sl]).astype(NP_BF16),
                "wv": np.ascontiguousarray(Wv[:, sl]).astype(NP_BF16),
                "wp": np.ascontiguousarray(Wp[sl, :]).astype(NP_BF16),
                "bq": np.ascontiguousarray(bq[sl]).astype(np.float32),
                "bk": np.ascontiguousarray(bk[sl]).astype(np.float32),
                "bv": np.ascontiguousarray(bv[sl]).astype(np.float32),
            }
        )
    return in_maps


def kernel(x, Wq, bq, Wk, bk, Wv, bv, Wp, bp, _results_hook=None, _trace=False):
    x = np.asarray(x, dtype=np.float32)
    nc = get_nc()
    in_maps = make_in_maps(x, Wq, bq, Wk, bk, Wv, bv, Wp, bp)
    res = run_bass_kernel_spmd(
        nc, in_maps, core_ids=list(range(N_CORES)), trace=_trace
    )
    if _results_hook is not None:
        _results_hook(res)
    out = np.zeros((B, T, C), dtype=np.float32)
    for core in range(N_CORES):
        b = core // GROUPS
        out[b] += res.results[core]["out"].astype(np.float32)
    out += np.asarray(bp, dtype=np.float32)[None, None, :]
    return out


# revision 10
# speedup vs baseline: 1.1720x; 1.0351x over previous
"""Causal self-attention Trainium2 kernel (8 NeuronCores, SPMD).

Sharding: 8 cores = 2 batches x 4 head-groups (4 heads of 64 dims each).
Each core computes full-sequence attention for its 4 heads plus the
partial output projection for its 256 y-columns; the host sums the 4
partials per batch and adds the output bias.

v2 design: single fused pipeline tuned to keep the PE continuously busy
(the HAM throttle on TRN2 re-engages after ~5us of PE idle, halving the
matmul rate, so PE gaps cost double):
  - per j-tile rounds: attT matmul chunks -> exp on ScalarE (both heads
    per instruction) -> yT accumulation, with projection / out-proj
    matmul quanta woven between chunks as PE fillers
  - PT (exp'd attention, transposed layout) kept in SBUF in a ragged
    causal layout (only j<=i trail per j-tile), written once, read once
  - q/k projection PSUM drains on ScalarE (Copy activation, fused bias;
    Copy shares the Exp activation table so no table reloads)
  - diagonal causal masks multiplied on GpSimd (Pool), off the DVE
  - softmax denominators via the ones-column trick (row 64 of yT PSUM);
    per-ib normalization with reciprocal_approx_fast + one selector
    matmul broadcasting 1/s to all 128 partitions
  - out[t, n] partial = yTn.T @ Wp, f32, DMA'd out per 128x512 tile
"""

import sys

for _p in ("/opt/trn_rl_repo",):
    if _p not in sys.path:
        sys.path.insert(0, _p)

from contextlib import ExitStack

import ml_dtypes
import numpy as np

import concourse.bass as bass
import concourse.tile as tile
from concourse import bacc, mybir
from concourse.bass_utils import run_bass_kernel_spmd

BF16 = mybir.dt.bfloat16
F32 = mybir.dt.float32
NP_BF16 = ml_dtypes.bfloat16

B, T, C = 2, 2048, 1024
H, D = 16, 64
N_CORES = 8
GROUPS = 4          # head groups (cores per batch)
MH = C // GROUPS    # 256 columns per core (4 heads)
LH = MH // D        # 4 local heads
CT = C // 128       # 8 contraction tiles
TT = T // 128       # 16 sequence tiles of 128
IB = T // 512       # 4 i-blocks of 512
SCALE = 1.0 / np.sqrt(D)

# ragged PT layout: trail(jt) = T - 128*jt columns, cumulative offsets
TRAIL = [T - 128 * jt for jt in range(TT)]
PT_OFF = [sum(TRAIL[:jt]) for jt in range(TT)]
PT_W = sum(TRAIL)   # 17408


def _selector() -> np.ndarray:
    """sel[r, l*64+j] = 1.0 if r == l else 0, [2, 128] bf16 — K=2 matmul
    broadcasts row lh of rs2 [2, 512] to out partitions 64*lh..64*lh+64."""
    sel = np.zeros((2, 128), dtype=NP_BF16)
    sel[0, 0:64] = 1.0
    sel[1, 64:128] = 1.0
    return sel


def _tri_mask() -> np.ndarray:
    """tri[j, c] = 1.0 if j <= c else 0 (bf16), [128, 128] — multiplicative
    causal mask for the diagonal 128x128 block of each j-tile."""
    j = np.arange(128)[:, None]
    c = np.arange(128)[None, :]
    return (j <= c).astype(NP_BF16)


def emit_kernel(
    nc, xT_d, wq_d, wk_d, wv_d, wp_d, bq_d, bk_d, bv_d, out_d, tri_d, sel_d
):
    with tile.TileContext(nc) as tc, ExitStack() as ctx:
        # ---- long-lived SBUF tiles --------------------------------------
        keep = ctx.enter_context(tc.tile_pool(name="keep", bufs=1))
        xT_s = keep.tile([128, CT, T], BF16, tag="xT")
        wq_s = keep.tile([128, CT, MH], BF16, tag="wq")
        wk_s = keep.tile([128, CT, MH], BF16, tag="wk")
        wv_s = keep.tile([128, CT, MH], BF16, tag="wv")
        wp_s = keep.tile([128, 2, C], BF16, tag="wp")
        qT_s = keep.tile([128, 2, T], BF16, tag="qT")
        kT_s = keep.tile([128, 2, T], BF16, tag="kT")
        v_s = keep.tile([128, TT, LH, D + 1], BF16, tag="v")
        PT_s = keep.tile([128, 2, PT_W], BF16, tag="PT")
        yTn_s = keep.tile([128, 2, T], BF16, tag="yTn")
        tri_st = keep.tile([128, 128], BF16, tag="tri_st")
        tri_s = keep.tile([128, 128], BF16, tag="tri")
        sel_st = keep.tile([2, 128], BF16, tag="sel_st")
        sel_s = keep.tile([2, 128], BF16, tag="sel")
        bq_st = keep.tile([128, 2], F32, tag="bq_st")
        bq_s = keep.tile([128, 2], F32, tag="bq")
        bk_st = keep.tile([128, 2], F32, tag="bk_st")
        bk_s = keep.tile([128, 2], F32, tag="bk")
        bv_row = keep.tile([1, MH], F32, tag="bv_row")
        bv_row_bf = keep.tile([1, MH], BF16, tag="bv_row_bf")
        bv_bc = keep.tile([128, MH], F32, tag="bv_bc")
        ones_bf128 = keep.tile([1, 128], BF16, tag="ones_bf128")

        # ---- pools ------------------------------------------------------
        att = ctx.enter_context(
            tc.tile_pool(name="att", bufs=2, space="PSUM"))      # 2x2 banks
        yps = ctx.enter_context(
            tc.tile_pool(name="yps", bufs=1, space="PSUM"))      # 2x1 bank
        mmp = ctx.enter_context(
            tc.tile_pool(name="mmp", bufs=2, space="PSUM"))      # 2 banks
        norm = ctx.enter_context(tc.tile_pool(name="norm", bufs=2))
        osb = ctx.enter_context(tc.tile_pool(name="osb", bufs=2))

        # ---- input DMAs (SP queue, first-needed first) ------------------
        xT_r = xT_d.ap().rearrange("(o p) t -> p o t", p=128)
        wq_r = wq_d.ap().rearrange("(o p) m -> p o m", p=128)
        wk_r = wk_d.ap().rearrange("(o p) m -> p o m", p=128)
        wv_r = wv_d.ap().rearrange("(o p) m -> p o m", p=128)
        wp_r = wp_d.ap().rearrange("(o p) n -> p o n", p=128)

        def xt_chunk(tb):
            nc.sync.dma_start(
                xT_s[:, :, tb * 512 : (tb + 1) * 512],
                xT_r[:, :, tb * 512 : (tb + 1) * 512],
            )

        nc.sync.dma_start(wq_s[:], wq_r[:])
        nc.sync.dma_start(wk_s[:], wk_r[:])
        xt_chunk(0)
        xt_chunk(1)
        xt_chunk(2)
        xt_chunk(3)
        nc.sync.dma_start(wv_s[:], wv_r[:])
        nc.sync.dma_start(wp_s[:], wp_r[:])
        # consts staged through a DVE copy: consumers then depend on DVE
        # program order instead of a DMA semaphore (walrus 1-wait limit)
        nc.gpsimd.dma_start(tri_st[:], tri_d.ap())
        nc.gpsimd.dma_start(sel_st[:], sel_d.ap())
        nc.gpsimd.dma_start(bq_st[:], bq_d.ap().rearrange("(o p) -> p o", p=128))
        nc.gpsimd.dma_start(bk_st[:], bk_d.ap().rearrange("(o p) -> p o", p=128))
        nc.gpsimd.dma_start(bv_row[:], bv_d.ap()[None, :])
        nc.vector.tensor_copy(tri_s[:], tri_st[:])
        nc.vector.tensor_copy(sel_s[:], sel_st[:])
        nc.vector.tensor_copy(bq_s[:], bq_st[:])
        nc.vector.tensor_copy(bk_s[:], bk_st[:])
        nc.vector.tensor_copy(bv_row_bf[:], bv_row[:])
        nc.vector.memset(ones_bf128[:], 1.0)
        nc.vector.memset(v_s[:, :, :, D : D + 1], 1.0)
        bv_ps = mmp.tile([128, 512], F32, tag="mm", name="bv_ps")
        nc.tensor.matmul(
            bv_ps[:, :MH], ones_bf128[:], bv_row_bf[:], start=True, stop=True
        )
        nc.vector.tensor_copy(bv_bc[:], bv_ps[:, :MH])

        # ---- emit helpers ----------------------------------------------
        def qk_block(w_s, b_s, dst, mt, tb, pool, pool_kw):
            """q/k projection for one (mt, tb): 8 matmuls + DVE drain (the
            ScalarE is the pacing engine during attention, keep it clear)."""
            ps = pool.tile([128, 512], F32, **pool_kw)
            for ct in range(CT):
                nc.tensor.matmul(
                    ps[:],
                    w_s[:, ct, mt * 128 : (mt + 1) * 128],
                    xT_s[:, ct, tb * 512 : (tb + 1) * 512],
                    start=(ct == 0),
                    stop=(ct == CT - 1),
                )
            nc.vector.tensor_scalar(
                dst[:, mt, tb * 512 : (tb + 1) * 512],
                ps[:],
                b_s[:, mt : mt + 1],
                None,
                mybir.AluOpType.add,
            )

        def v_block(tt):
            """v projection for one tt: 8 matmuls + DVE drain (bias add)."""
            ps = mmp.tile([128, 512], F32, tag="mm", name="v_ps")
            for ct in range(CT):
                nc.tensor.matmul(
                    ps[:, :MH],
                    xT_s[:, ct, tt * 128 : (tt + 1) * 128],
                    wv_s[:, ct, :],
                    start=(ct == 0),
                    stop=(ct == CT - 1),
                )
            nc.vector.tensor_tensor(
                v_s[:, tt, :, 0:D],
                ps[:, :MH].rearrange("p (h d) -> p h d", h=LH),
                bv_bc[:].rearrange("p (h d) -> p h d", h=LH),
                mybir.AluOpType.add,
            )

        in_tail = [False]
        ostate = {"n": 0, "ot": None}

        def out_block(tt, nb):
            """out projection for one (tt, nb): 2 matmuls + drain; full
            [128, 1024] DMA per tt (coalesced across nb). PSUM rotates
            mm/sps (att pool in tail); drains alternate DVE/ScalarE;
            DMA queues alternate sync/gpsimd."""
            k = ostate["n"]
            ostate["n"] += 1
            if in_tail[0]:
                ab = att.tile([128, 2, 512], F32, tag="att", name="o_ps")
                ps = ab[:, 0, :]
            else:
                ps = mmp.tile([128, 512], F32, tag="mm", name="o_ps")
            for pp in range(2):
                nc.tensor.matmul(
                    ps,
                    yTn_s[:, pp, tt * 128 : (tt + 1) * 128],
                    wp_s[:, pp, nb * 512 : (nb + 1) * 512],
                    start=(pp == 0),
                    stop=(pp == 1),
                )
            if nb == 0:
                ostate["ot"] = osb.tile(
                    [128, 1024], BF16, tag="out_t", name="out_t"
                )
            ot = ostate["ot"]
            if in_tail[0]:
                nc.scalar.activation(
                    ot[:, nb * 512 : (nb + 1) * 512],
                    ps,
                    mybir.ActivationFunctionType.Copy,
                )
            else:
                nc.vector.tensor_copy(ot[:, nb * 512 : (nb + 1) * 512], ps)
            if nb == 1:
                eng = nc.sync if (tt % 2 == 0) else nc.gpsimd
                eng.dma_start(out_r[tt, :, :], ot[:])

        out_r = out_d.ap().rearrange("(tt p) n -> tt p n", p=128)

        # filler queue: list of zero-arg closures, each one PE quantum
        fillers = []

        def pump(n=1):
            for _ in range(n):
                if fillers:
                    fillers.pop(0)()

        # ---- phase A: q/k projections for pair 0 (att pool as psum) ----
        for tb in range(IB):
            ab = att.tile([128, 2, 512], F32, tag="att", name="ab")
            # use the two halves of an att tile for q and k drains
            for half, (w_s, b_s, dst) in enumerate(
                ((wq_s, bq_s, qT_s), (wk_s, bk_s, kT_s))
            ):
                for ct in range(CT):
                    nc.tensor.matmul(
                        ab[:, half, :],
                        w_s[:, ct, 0:128],
                        xT_s[:, ct, tb * 512 : (tb + 1) * 512],
                        start=(ct == 0),
                        stop=(ct == CT - 1),
                    )
                nc.scalar.activation(
                    dst[:, 0, tb * 512 : (tb + 1) * 512],
                    ab[:, half, :],
                    mybir.ActivationFunctionType.Identity,
                    bias=b_s[:, 0:1],
                )

        # fillers for pair-0 attention phase: v blocks + q/k mt=1 blocks
        for tt in range(TT):
            fillers.append(lambda tt=tt: v_block(tt))
            if tt % 2 == 1:
                tb = tt // 2
                if tb < IB:
                    fillers.append(
                        lambda tb=tb: qk_block(
                            wq_s, bq_s, qT_s, 1, tb, mmp, dict(tag="mm", name="q_ps")
                        )
                    )
                    fillers.append(
                        lambda tb=tb: qk_block(
                            wk_s, bk_s, kT_s, 1, tb, mmp, dict(tag="mm", name="k_ps")
                        )
                    )

        # ---- attention pairs --------------------------------------------
        def attT_trail(p, jt):
            """attT + exp + diag mask for j-tile jt, full causal trail."""
            ia = 128 * jt
            trail = TRAIL[jt]
            off = PT_OFF[jt]
            c = 0
            while c < trail:
                n = min(512, trail - c)
                ab = att.tile([128, 2, 512], F32, tag="att", name="ab")
                for lh in range(2):
                    prow = slice(64 * lh, 64 * lh + 64)
                    nc.tensor.matmul(
                        ab[:, lh, :n],
                        kT_s[prow, p, ia : ia + 128],
                        qT_s[prow, p, ia + c : ia + c + n],
                        start=True,
                        stop=True,
                    )
                pump(1)
                nc.scalar.activation(
                    PT_s[:, :, off + c : off + c + n],
                    ab[:, :, :n],
                    mybir.ActivationFunctionType.Exp,
                    scale=float(SCALE),
                )
                c += n
            # diagonal causal mask on GpSimd (Pool), off the DVE
            for lh in range(2):
                nc.gpsimd.tensor_tensor(
                    PT_s[:, lh, off : off + 128],
                    PT_s[:, lh, off : off + 128],
                    tri_s[:],
                    mybir.AluOpType.mult,
                )

        for p in range(2):
            for ib in range(IB):
                # yT accumulation interleaved with the 4 new j-tiles' attT
                # trails (emitted one j-tile ahead so exp + mask are done
                # before the consuming yT matmul; ScalarE stays fed during
                # the yT burst instead of starving then refilling cold)
                yts = [
                    yps.tile([D + 1, 512], F32, tag=f"y{lh}", name=f"yt{lh}")
                    for lh in range(2)
                ]
                attT_trail(p, 4 * ib)
                for jt2 in range(4 * ib + 4):
                    if 4 * ib <= jt2 <= 4 * ib + 2:
                        attT_trail(p, jt2 + 1)
                    ia2 = 128 * jt2
                    c0 = max(512 * ib, ia2)
                    w = 512 * ib + 512 - c0
                    for lh in range(2):
                        nc.tensor.matmul(
                            yts[lh][:, c0 - 512 * ib : 512],
                            v_s[:, jt2, 2 * p + lh, :],
                            PT_s[:, lh, PT_OFF[jt2] + c0 - ia2 :
                                 PT_OFF[jt2] + c0 - ia2 + w],
                            start=(jt2 == 0),
                            stop=(jt2 == 4 * ib + 3),
                        )
                    if jt2 % 2 == 0:
                        pump(1)

                # drain + normalize this ib
                srows = norm.tile([2, 512], F32, tag="srows", name="srows")
                rs2f = norm.tile([2, 512], F32, tag="rs2f", name="rs2f")
                rs2 = norm.tile([2, 512], BF16, tag="rs2", name="rs2")
                yTall = [
                    norm.tile([D + 1, 512], F32, tag=f"yTall{lh}", name=f"yTall{lh}")
                    for lh in range(2)
                ]
                for lh in range(2):
                    nc.vector.tensor_copy(yTall[lh][:], yts[lh][:])
                    nc.sync.dma_start(srows[lh : lh + 1, :], yTall[lh][D : D + 1, :])
                nc.vector.reciprocal_approx_fast(rs2f[:], srows[:])
                nc.vector.tensor_copy(rs2[:], rs2f[:])
                pump(1)
                S_ps = mmp.tile([128, 512], F32, tag="mm", name="S_ps")
                nc.tensor.matmul(S_ps[:], sel_s[:], rs2[:], start=True, stop=True)
                for lh in range(2):
                    nc.vector.tensor_tensor(
                        yTn_s[64 * lh : 64 * lh + 64, p, 512 * ib : 512 * ib + 512],
                        yTall[lh][0:D, :],
                        S_ps[64 * lh : 64 * lh + 64, :],
                        mybir.AluOpType.mult,
                    )
                if p == 1:
                    # both pairs normalized for this ib: queue out-proj fillers
                    for tt in range(4 * ib, 4 * ib + 4):
                        for nb in range(2):
                            fillers.append(
                                lambda tt=tt, nb=nb: out_block(tt, nb)
                            )

        # tail: drain remaining fillers (last ib's out-proj) on att pool
        in_tail[0] = True
        while fillers:
            f = fillers.pop(0)
            f()


_NC_CACHE = None


def get_nc() -> bass.Bass:
    global _NC_CACHE
    if _NC_CACHE is None:
        nc = bacc.Bacc()
        xT_d = nc.declare_dram_parameter("xT", [C, T], BF16, isOutput=False)
        wq_d = nc.declare_dram_parameter("wq", [C, MH], BF16, isOutput=False)
        wk_d = nc.declare_dram_parameter("wk", [C, MH], BF16, isOutput=False)
        wv_d = nc.declare_dram_parameter("wv", [C, MH], BF16, isOutput=False)
        wp_d = nc.declare_dram_parameter("wp", [MH, C], BF16, isOutput=False)
        bq_d = nc.declare_dram_parameter("bq", [MH], F32, isOutput=False)
        bk_d = nc.declare_dram_parameter("bk", [MH], F32, isOutput=False)
        bv_d = nc.declare_dram_parameter("bv", [MH], F32, isOutput=False)
        out_d = nc.declare_dram_parameter("out", [T, C], BF16, isOutput=True)
        tri_d = nc.inline_tensor(_tri_mask(), name="tri_mask")
        sel_d = nc.inline_tensor(_selector(), name="selector")
        emit_kernel(
            nc, xT_d, wq_d, wk_d, wv_d, wp_d, bq_d, bk_d, bv_d, out_d, tri_d, sel_d
        )
        nc.finalize()
        _NC_CACHE = nc
    return _NC_CACHE


def make_in_maps(x, Wq, bq, Wk, bk, Wv, bv, Wp, bp):
    in_maps = []
    for core in range(N_CORES):
        b, g = divmod(core, GROUPS)
        sl = slice(g * MH, (g + 1) * MH)
        in_maps.append(
            {
                "xT": np.ascontiguousarray(x[b].T).astype(NP_BF16),
                "wq": np.ascontiguousarray(Wq[:, sl]).astype(NP_BF16),
                "wk": np.ascontiguousarray(Wk[:, sl]).astype(NP_BF16),
                "wv": np.ascontiguousarray(Wv[:, sl]).astype(NP_BF16),
                "wp": np.ascontiguousarray(Wp[sl, :]).astype(NP_BF16),
                "bq": np.ascontiguousarray(bq[sl]).astype(np.float32),
                "bk": np.ascontiguousarray(bk[sl]).astype(np.float32),
                "bv": np.ascontiguousarray(bv[sl]).astype(np.float32),
            }
        )
    return in_maps


def kernel(x, Wq, bq, Wk, bk, Wv, bv, Wp, bp, _results_hook=None, _trace=False):
    x = np.asarray(x, dtype=np.float32)
    nc = get_nc()
    in_maps = make_in_maps(x, Wq, bq, Wk, bk, Wv, bv, Wp, bp)
    res = run_bass_kernel_spmd(
        nc, in_maps, core_ids=list(range(N_CORES)), trace=_trace
    )
    if _results_hook is not None:
        _results_hook(res)
    out = np.zeros((B, T, C), dtype=np.float32)
    for core in range(N_CORES):
        b = core // GROUPS
        out[b] += res.results[core]["out"].astype(np.float32)
    out += np.asarray(bp, dtype=np.float32)[None, None, :]
    return out


# revision 11
# speedup vs baseline: 1.2120x; 1.0341x over previous
"""Causal self-attention Trainium2 kernel (8 NeuronCores, SPMD).

Sharding: 8 cores = 2 batches x 4 head-groups (4 heads of 64 dims each).
Each core computes full-sequence attention for its 4 heads plus the
partial output projection for its 256 y-columns; the host sums the 4
partials per batch and adds the output bias.

v2 design: single fused pipeline tuned to keep the PE continuously busy
(the HAM throttle on TRN2 re-engages after ~5us of PE idle, halving the
matmul rate, so PE gaps cost double):
  - per j-tile rounds: attT matmul chunks -> exp on ScalarE (both heads
    per instruction) -> yT accumulation, with projection / out-proj
    matmul quanta woven between chunks as PE fillers
  - PT (exp'd attention, transposed layout) kept in SBUF in a ragged
    causal layout (only j<=i trail per j-tile), written once, read once
  - q/k projection PSUM drains on ScalarE (Copy activation, fused bias;
    Copy shares the Exp activation table so no table reloads)
  - diagonal causal masks multiplied on GpSimd (Pool), off the DVE
  - softmax denominators via the ones-column trick (row 64 of yT PSUM);
    per-ib normalization with reciprocal_approx_fast + one selector
    matmul broadcasting 1/s to all 128 partitions
  - out[t, n] partial = yTn.T @ Wp, f32, DMA'd out per 128x512 tile
"""

import sys

for _p in ("/opt/trn_rl_repo",):
    if _p not in sys.path:
        sys.path.insert(0, _p)

from contextlib import ExitStack

import ml_dtypes
import numpy as np

import concourse.bass as bass
import concourse.tile as tile
from concourse import bacc, mybir
from concourse.bass_utils import run_bass_kernel_spmd

BF16 = mybir.dt.bfloat16
F32 = mybir.dt.float32
NP_BF16 = ml_dtypes.bfloat16

B, T, C = 2, 2048, 1024
H, D = 16, 64
N_CORES = 8
GROUPS = 4          # head groups (cores per batch)
MH = C // GROUPS    # 256 columns per core (4 heads)
LH = MH // D        # 4 local heads
CT = C // 128       # 8 contraction tiles
TT = T // 128       # 16 sequence tiles of 128
IB = T // 512       # 4 i-blocks of 512
SCALE = 1.0 / np.sqrt(D)

# ragged PT layout: trail(jt) = T - 128*jt columns, cumulative offsets
TRAIL = [T - 128 * jt for jt in range(TT)]
PT_OFF = [sum(TRAIL[:jt]) for jt in range(TT)]
PT_W = sum(TRAIL)   # 17408


def _selector() -> np.ndarray:
    """sel[r, l*64+j] = 1.0 if r == l else 0, [2, 128] bf16 — K=2 matmul
    broadcasts row lh of rs2 [2, 512] to out partitions 64*lh..64*lh+64."""
    sel = np.zeros((2, 128), dtype=NP_BF16)
    sel[0, 0:64] = 1.0
    sel[1, 64:128] = 1.0
    return sel


def _tri_mask() -> np.ndarray:
    """tri[j, c] = 1.0 if j <= c else 0 (bf16), [128, 128] — multiplicative
    causal mask for the diagonal 128x128 block of each j-tile."""
    j = np.arange(128)[:, None]
    c = np.arange(128)[None, :]
    return (j <= c).astype(NP_BF16)


def emit_kernel(
    nc, xT_d, wq_d, wk_d, wv_d, wp_d, bq_d, bk_d, bv_d, out_d, tri_d, sel_d
):
    with tile.TileContext(nc) as tc, ExitStack() as ctx:
        # ---- long-lived SBUF tiles --------------------------------------
        keep = ctx.enter_context(tc.tile_pool(name="keep", bufs=1))
        xT_s = keep.tile([128, CT, T], BF16, tag="xT")
        wq_s = keep.tile([128, CT, MH], BF16, tag="wq")
        wk_s = keep.tile([128, CT, MH], BF16, tag="wk")
        wv_s = keep.tile([128, CT, MH], BF16, tag="wv")
        wp_s = keep.tile([128, 2, C], BF16, tag="wp")
        qT_s = keep.tile([128, 2, T], BF16, tag="qT")
        kT_s = keep.tile([128, 2, T], BF16, tag="kT")
        v_s = keep.tile([128, TT, LH, D + 1], BF16, tag="v")
        PT_s = keep.tile([128, 2, PT_W], BF16, tag="PT")
        yTn_s = keep.tile([128, 2, T], BF16, tag="yTn")
        tri_st = keep.tile([128, 128], BF16, tag="tri_st")
        tri_s = keep.tile([128, 128], BF16, tag="tri")
        sel_st = keep.tile([2, 128], BF16, tag="sel_st")
        sel_s = keep.tile([2, 128], BF16, tag="sel")
        bq_st = keep.tile([128, 2], F32, tag="bq_st")
        bq_s = keep.tile([128, 2], F32, tag="bq")
        bk_st = keep.tile([128, 2], F32, tag="bk_st")
        bk_s = keep.tile([128, 2], F32, tag="bk")
        bv_row = keep.tile([1, MH], F32, tag="bv_row")
        bv_row_bf = keep.tile([1, MH], BF16, tag="bv_row_bf")
        bv_bc = keep.tile([128, MH], F32, tag="bv_bc")
        ones_bf128 = keep.tile([1, 128], BF16, tag="ones_bf128")

        # ---- pools ------------------------------------------------------
        att = ctx.enter_context(
            tc.tile_pool(name="att", bufs=2, space="PSUM"))      # 2x2 banks
        yps = ctx.enter_context(
            tc.tile_pool(name="yps", bufs=1, space="PSUM"))      # 2x1 bank
        mmp = ctx.enter_context(
            tc.tile_pool(name="mmp", bufs=2, space="PSUM"))      # 2 banks
        norm = ctx.enter_context(tc.tile_pool(name="norm", bufs=2))
        osb = ctx.enter_context(tc.tile_pool(name="osb", bufs=2))

        # ---- input DMAs (SP queue, first-needed first) ------------------
        xT_r = xT_d.ap().rearrange("(o p) t -> p o t", p=128)
        wq_r = wq_d.ap().rearrange("(o p) m -> p o m", p=128)
        wk_r = wk_d.ap().rearrange("(o p) m -> p o m", p=128)
        wv_r = wv_d.ap().rearrange("(o p) m -> p o m", p=128)
        wp_r = wp_d.ap().rearrange("(o p) n -> p o n", p=128)

        def xt_chunk(tb):
            nc.sync.dma_start(
                xT_s[:, :, tb * 512 : (tb + 1) * 512],
                xT_r[:, :, tb * 512 : (tb + 1) * 512],
            )

        nc.sync.dma_start(wq_s[:], wq_r[:])
        nc.sync.dma_start(wk_s[:], wk_r[:])
        xt_chunk(0)
        xt_chunk(1)
        xt_chunk(2)
        xt_chunk(3)
        nc.sync.dma_start(wv_s[:], wv_r[:])
        nc.sync.dma_start(wp_s[:], wp_r[:])
        # consts staged through a DVE copy: consumers then depend on DVE
        # program order instead of a DMA semaphore (walrus 1-wait limit)
        nc.gpsimd.dma_start(tri_st[:], tri_d.ap())
        nc.gpsimd.dma_start(sel_st[:], sel_d.ap())
        nc.gpsimd.dma_start(bq_st[:], bq_d.ap().rearrange("(o p) -> p o", p=128))
        nc.gpsimd.dma_start(bk_st[:], bk_d.ap().rearrange("(o p) -> p o", p=128))
        nc.gpsimd.dma_start(bv_row[:], bv_d.ap()[None, :])
        nc.vector.tensor_copy(tri_s[:], tri_st[:])
        nc.vector.tensor_copy(sel_s[:], sel_st[:])
        nc.vector.tensor_copy(bq_s[:], bq_st[:])
        nc.vector.tensor_copy(bk_s[:], bk_st[:])
        nc.vector.tensor_copy(bv_row_bf[:], bv_row[:])
        nc.vector.memset(ones_bf128[:], 1.0)
        nc.vector.memset(v_s[:, :, :, D : D + 1], 1.0)
        bv_ps = mmp.tile([128, 512], F32, tag="mm", name="bv_ps")
        nc.tensor.matmul(
            bv_ps[:, :MH], ones_bf128[:], bv_row_bf[:], start=True, stop=True
        )
        nc.vector.tensor_copy(bv_bc[:], bv_ps[:, :MH])

        # ---- emit helpers ----------------------------------------------
        def qk_block(w_s, b_s, dst, mt, tb, pool, pool_kw):
            """q/k projection for one (mt, tb): 8 matmuls + DVE drain (the
            ScalarE is the pacing engine during attention, keep it clear)."""
            ps = pool.tile([128, 512], F32, **pool_kw)
            for ct in range(CT):
                nc.tensor.matmul(
                    ps[:],
                    w_s[:, ct, mt * 128 : (mt + 1) * 128],
                    xT_s[:, ct, tb * 512 : (tb + 1) * 512],
                    start=(ct == 0),
                    stop=(ct == CT - 1),
                )
            nc.vector.tensor_scalar(
                dst[:, mt, tb * 512 : (tb + 1) * 512],
                ps[:],
                b_s[:, mt : mt + 1],
                None,
                mybir.AluOpType.add,
            )

        def v_block(tt):
            """v projection for one tt: 8 matmuls + DVE drain (bias add)."""
            ps = mmp.tile([128, 512], F32, tag="mm", name="v_ps")
            for ct in range(CT):
                nc.tensor.matmul(
                    ps[:, :MH],
                    xT_s[:, ct, tt * 128 : (tt + 1) * 128],
                    wv_s[:, ct, :],
                    start=(ct == 0),
                    stop=(ct == CT - 1),
                )
            nc.vector.tensor_tensor(
                v_s[:, tt, :, 0:D],
                ps[:, :MH].rearrange("p (h d) -> p h d", h=LH),
                bv_bc[:].rearrange("p (h d) -> p h d", h=LH),
                mybir.AluOpType.add,
            )

        in_tail = [False]
        ostate = {"n": 0, "ot": None}

        def out_block(tt, nb):
            """out projection for one (tt, nb): 2 matmuls + drain; full
            [128, 1024] DMA per tt (coalesced across nb). PSUM rotates
            mm/sps (att pool in tail); drains alternate DVE/ScalarE;
            DMA queues alternate sync/gpsimd."""
            k = ostate["n"]
            ostate["n"] += 1
            if in_tail[0]:
                ab = att.tile([128, 2, 512], F32, tag="att", name="o_ps")
                ps = ab[:, 0, :]
            else:
                ps = mmp.tile([128, 512], F32, tag="mm", name="o_ps")
            for pp in range(2):
                nc.tensor.matmul(
                    ps,
                    yTn_s[:, pp, tt * 128 : (tt + 1) * 128],
                    wp_s[:, pp, nb * 512 : (nb + 1) * 512],
                    start=(pp == 0),
                    stop=(pp == 1),
                )
            if nb == 0:
                ostate["ot"] = osb.tile(
                    [128, 1024], BF16, tag="out_t", name="out_t"
                )
            ot = ostate["ot"]
            if in_tail[0]:
                nc.scalar.activation(
                    ot[:, nb * 512 : (nb + 1) * 512],
                    ps,
                    mybir.ActivationFunctionType.Copy,
                )
            else:
                nc.vector.tensor_copy(ot[:, nb * 512 : (nb + 1) * 512], ps)
            if nb == 1:
                eng = nc.sync if (tt % 2 == 0) else nc.gpsimd
                eng.dma_start(out_r[tt, :, :], ot[:])

        out_r = out_d.ap().rearrange("(tt p) n -> tt p n", p=128)

        # filler queue: list of zero-arg closures, each one PE quantum
        fillers = []

        def pump(n=1):
            for _ in range(n):
                if fillers:
                    fillers.pop(0)()

        # ---- phase A: q/k projections for pair 0 (att pool as psum) ----
        for tb in range(IB):
            ab = att.tile([128, 2, 512], F32, tag="att", name="ab")
            # use the two halves of an att tile for q and k drains
            for half, (w_s, b_s, dst) in enumerate(
                ((wq_s, bq_s, qT_s), (wk_s, bk_s, kT_s))
            ):
                for ct in range(CT):
                    nc.tensor.matmul(
                        ab[:, half, :],
                        w_s[:, ct, 0:128],
                        xT_s[:, ct, tb * 512 : (tb + 1) * 512],
                        start=(ct == 0),
                        stop=(ct == CT - 1),
                    )
                nc.scalar.activation(
                    dst[:, 0, tb * 512 : (tb + 1) * 512],
                    ab[:, half, :],
                    mybir.ActivationFunctionType.Identity,
                    bias=b_s[:, 0:1],
                )

        # fillers for pair-0 attention phase: v blocks + q/k mt=1 blocks
        for tt in range(TT):
            fillers.append(lambda tt=tt: v_block(tt))
            if tt % 2 == 1:
                tb = tt // 2
                if tb < IB:
                    fillers.append(
                        lambda tb=tb: qk_block(
                            wq_s, bq_s, qT_s, 1, tb, mmp, dict(tag="mm", name="q_ps")
                        )
                    )
                    fillers.append(
                        lambda tb=tb: qk_block(
                            wk_s, bk_s, kT_s, 1, tb, mmp, dict(tag="mm", name="k_ps")
                        )
                    )

        # ---- attention pairs --------------------------------------------
        def attT_trail(p, jt):
            """attT + exp + diag mask for j-tile jt, full causal trail."""
            ia = 128 * jt
            trail = TRAIL[jt]
            off = PT_OFF[jt]
            c = 0
            while c < trail:
                n = min(512, trail - c)
                ab = att.tile([128, 2, 512], F32, tag="att", name="ab")
                for lh in range(2):
                    prow = slice(64 * lh, 64 * lh + 64)
                    nc.tensor.matmul(
                        ab[:, lh, :n],
                        kT_s[prow, p, ia : ia + 128],
                        qT_s[prow, p, ia + c : ia + c + n],
                        start=True,
                        stop=True,
                    )
                pump(1)
                nc.scalar.activation(
                    PT_s[:, :, off + c : off + c + n],
                    ab[:, :, :n],
                    mybir.ActivationFunctionType.Exp,
                    scale=float(SCALE),
                )
                c += n
            # diagonal causal mask on GpSimd (Pool), off the DVE
            for lh in range(2):
                nc.gpsimd.tensor_tensor(
                    PT_s[:, lh, off : off + 128],
                    PT_s[:, lh, off : off + 128],
                    tri_s[:],
                    mybir.AluOpType.mult,
                )

        for p in range(2):
            for ib in range(IB):
                # 4 new j-tiles' attT trails first, then the yT burst
                # (interleaving trails into the burst measured slower:
                # PSUM-group switching breaks PE pipelining)
                for r in range(4):
                    attT_trail(p, 4 * ib + r)
                yts = [
                    yps.tile([D + 1, 512], F32, tag=f"y{lh}", name=f"yt{lh}")
                    for lh in range(2)
                ]
                for jt2 in range(4 * ib + 4):
                    ia2 = 128 * jt2
                    c0 = max(512 * ib, ia2)
                    w = 512 * ib + 512 - c0
                    for lh in range(2):
                        nc.tensor.matmul(
                            yts[lh][:, c0 - 512 * ib : 512],
                            v_s[:, jt2, 2 * p + lh, :],
                            PT_s[:, lh, PT_OFF[jt2] + c0 - ia2 :
                                 PT_OFF[jt2] + c0 - ia2 + w],
                            start=(jt2 == 0),
                            stop=(jt2 == 4 * ib + 3),
                        )
                    if jt2 % 2 == 0:
                        pump(1)

                # drain + normalize this ib
                srows = norm.tile([2, 512], F32, tag="srows", name="srows")
                rs2f = norm.tile([2, 512], F32, tag="rs2f", name="rs2f")
                rs2 = norm.tile([2, 512], BF16, tag="rs2", name="rs2")
                yTall = [
                    norm.tile([D + 1, 512], F32, tag=f"yTall{lh}", name=f"yTall{lh}")
                    for lh in range(2)
                ]
                for lh in range(2):
                    nc.vector.tensor_copy(yTall[lh][:], yts[lh][:])
                    nc.sync.dma_start(srows[lh : lh + 1, :], yTall[lh][D : D + 1, :])
                nc.vector.reciprocal_approx_fast(rs2f[:], srows[:])
                nc.vector.tensor_copy(rs2[:], rs2f[:])
                pump(1)
                S_ps = mmp.tile([128, 512], F32, tag="mm", name="S_ps")
                nc.tensor.matmul(S_ps[:], sel_s[:], rs2[:], start=True, stop=True)
                for lh in range(2):
                    nc.vector.tensor_tensor(
                        yTn_s[64 * lh : 64 * lh + 64, p, 512 * ib : 512 * ib + 512],
                        yTall[lh][0:D, :],
                        S_ps[64 * lh : 64 * lh + 64, :],
                        mybir.AluOpType.mult,
                    )
                if p == 1:
                    # both pairs normalized for this ib: queue out-proj fillers
                    for tt in range(4 * ib, 4 * ib + 4):
                        for nb in range(2):
                            fillers.append(
                                lambda tt=tt, nb=nb: out_block(tt, nb)
                            )

        # tail: drain remaining fillers (last ib's out-proj) on att pool
        in_tail[0] = True
        while fillers:
            f = fillers.pop(0)
            f()


_NC_CACHE = None


def get_nc() -> bass.Bass:
    global _NC_CACHE
    if _NC_CACHE is None:
        nc = bacc.Bacc()
        xT_d = nc.declare_dram_parameter("xT", [C, T], BF16, isOutput=False)
        wq_d = nc.declare_dram_parameter("wq", [C, MH], BF16, isOutput=False)
        wk_d = nc.declare_dram_parameter("wk", [C, MH], BF16, isOutput=False)
        wv_d = nc.declare_dram_parameter("wv", [C, MH], BF16, isOutput=False)
        wp_d = nc.declare_dram_parameter("wp", [MH, C], BF16, isOutput=False)
        bq_d = nc.declare_dram_parameter("bq", [MH], F32, isOutput=False)
        bk_d = nc.declare_dram_parameter("bk", [MH], F32, isOutput=False)
        bv_d = nc.declare_dram_parameter("bv", [MH], F32, isOutput=False)
        out_d = nc.declare_dram_parameter("out", [T, C], BF16, isOutput=True)
        tri_d = nc.inline_tensor(_tri_mask(), name="tri_mask")
        sel_d = nc.inline_tensor(_selector(), name="selector")
        emit_kernel(
            nc, xT_d, wq_d, wk_d, wv_d, wp_d, bq_d, bk_d, bv_d, out_d, tri_d, sel_d
        )
        nc.finalize()
        _NC_CACHE = nc
    return _NC_CACHE


def make_in_maps(x, Wq, bq, Wk, bk, Wv, bv, Wp, bp):
    in_maps = []
    for core in range(N_CORES):
        b, g = divmod(core, GROUPS)
        sl = slice(g * MH, (g + 1) * MH)
        in_maps.append(
            {
                "xT": np.ascontiguousarray(x[b].T).astype(NP_BF16),
                "wq": np.ascontiguousarray(Wq[:, sl]).astype(NP_BF16),
                "wk": np.ascontiguousarray(Wk[:, sl]).astype(NP_BF16),
                "wv": np.ascontiguousarray(Wv[:, sl]).astype(NP_BF16),
                "wp": np.ascontiguousarray(Wp[sl, :]).astype(NP_BF16),
                "bq": np.ascontiguousarray(bq[sl]).astype(np.float32),
                "bk": np.ascontiguousarray(bk[sl]).astype(np.float32),
                "bv": np.ascontiguousarray(bv[sl]).astype(np.float32),
            }
        )
    return in_maps


def kernel(x, Wq, bq, Wk, bk, Wv, bv, Wp, bp, _results_hook=None, _trace=False):
    x = np.asarray(x, dtype=np.float32)
    nc = get_nc()
    in_maps = make_in_maps(x, Wq, bq, Wk, bk, Wv, bv, Wp, bp)
    res = run_bass_kernel_spmd(
        nc, in_maps, core_ids=list(range(N_CORES)), trace=_trace
    )
    if _results_hook is not None:
        _results_hook(res)
    out = np.zeros((B, T, C), dtype=np.float32)
    for core in range(N_CORES):
        b = core // GROUPS
        out[b] += res.results[core]["out"].astype(np.float32)
    out += np.asarray(bp, dtype=np.float32)[None, None, :]
    return out


# revision 12
# speedup vs baseline: 1.2222x; 1.0084x over previous
"""Causal self-attention Trainium2 kernel (8 NeuronCores, SPMD).

Sharding: 8 cores = 2 batches x 4 head-groups (4 heads of 64 dims each).
Each core computes full-sequence attention for its 4 heads plus the
partial output projection for its 256 y-columns; the host sums the 4
partials per batch and adds the output bias.

v2 design: single fused pipeline tuned to keep the PE continuously busy
(the HAM throttle on TRN2 re-engages after ~5us of PE idle, halving the
matmul rate, so PE gaps cost double):
  - per j-tile rounds: attT matmul chunks -> exp on ScalarE (both heads
    per instruction) -> yT accumulation, with projection / out-proj
    matmul quanta woven between chunks as PE fillers
  - PT (exp'd attention, transposed layout) kept in SBUF in a ragged
    causal layout (only j<=i trail per j-tile), written once, read once
  - q/k projection PSUM drains on ScalarE (Copy activation, fused bias;
    Copy shares the Exp activation table so no table reloads)
  - diagonal causal masks multiplied on GpSimd (Pool), off the DVE
  - softmax denominators via the ones-column trick (row 64 of yT PSUM);
    per-ib normalization with reciprocal_approx_fast + one selector
    matmul broadcasting 1/s to all 128 partitions
  - out[t, n] partial = yTn.T @ Wp, f32, DMA'd out per 128x512 tile
"""

import sys

for _p in ("/opt/trn_rl_repo",):
    if _p not in sys.path:
        sys.path.insert(0, _p)

from contextlib import ExitStack

import ml_dtypes
import numpy as np

import concourse.bass as bass
import concourse.tile as tile
from concourse import bacc, mybir
from concourse.bass_utils import run_bass_kernel_spmd

BF16 = mybir.dt.bfloat16
F32 = mybir.dt.float32
NP_BF16 = ml_dtypes.bfloat16

B, T, C = 2, 2048, 1024
H, D = 16, 64
N_CORES = 8
GROUPS = 4          # head groups (cores per batch)
MH = C // GROUPS    # 256 columns per core (4 heads)
LH = MH // D        # 4 local heads
CT = C // 128       # 8 contraction tiles
TT = T // 128       # 16 sequence tiles of 128
IB = T // 512       # 4 i-blocks of 512
SCALE = 1.0 / np.sqrt(D)

# ragged PT layout: trail(jt) = T - 128*jt columns, cumulative offsets
TRAIL = [T - 128 * jt for jt in range(TT)]
PT_OFF = [sum(TRAIL[:jt]) for jt in range(TT)]
PT_W = sum(TRAIL)   # 17408


def _selector() -> np.ndarray:
    """sel[r, l*64+j] = 1.0 if r == l else 0, [2, 128] bf16 — K=2 matmul
    broadcasts row lh of rs2 [2, 512] to out partitions 64*lh..64*lh+64."""
    sel = np.zeros((2, 128), dtype=NP_BF16)
    sel[0, 0:64] = 1.0
    sel[1, 64:128] = 1.0
    return sel


def _tri_mask() -> np.ndarray:
    """tri[j, c] = 1.0 if j <= c else 0 (bf16), [128, 128] — multiplicative
    causal mask for the diagonal 128x128 block of each j-tile."""
    j = np.arange(128)[:, None]
    c = np.arange(128)[None, :]
    return (j <= c).astype(NP_BF16)


def emit_kernel(
    nc, xT_d, wq_d, wk_d, wv_d, wp_d, bq_d, bk_d, bv_d, out_d, tri_d, sel_d
):
    with tile.TileContext(nc) as tc, ExitStack() as ctx:
        # ---- long-lived SBUF tiles --------------------------------------
        keep = ctx.enter_context(tc.tile_pool(name="keep", bufs=1))
        xT_s = keep.tile([128, CT, T], BF16, tag="xT")
        wq_s = keep.tile([128, CT, MH], BF16, tag="wq")
        wk_s = keep.tile([128, CT, MH], BF16, tag="wk")
        wv_s = keep.tile([128, CT, MH], BF16, tag="wv")
        wp_s = keep.tile([128, 2, C], BF16, tag="wp")
        qT_s = keep.tile([128, 2, T], BF16, tag="qT")
        kT_s = keep.tile([128, 2, T], BF16, tag="kT")
        v_s = keep.tile([128, TT, LH, D + 1], BF16, tag="v")
        PT_s = keep.tile([128, 2, PT_W], BF16, tag="PT")
        yTn_s = keep.tile([128, 2, T], BF16, tag="yTn")
        tri_st = keep.tile([128, 128], BF16, tag="tri_st")
        tri_s = keep.tile([128, 128], BF16, tag="tri")
        sel_st = keep.tile([2, 128], BF16, tag="sel_st")
        sel_s = keep.tile([2, 128], BF16, tag="sel")
        bq_st = keep.tile([128, 2], F32, tag="bq_st")
        bq_s = keep.tile([128, 2], F32, tag="bq")
        bk_st = keep.tile([128, 2], F32, tag="bk_st")
        bk_s = keep.tile([128, 2], F32, tag="bk")
        bv_row = keep.tile([1, MH], F32, tag="bv_row")
        bv_row_bf = keep.tile([1, MH], BF16, tag="bv_row_bf")
        bv_bc = keep.tile([128, MH], F32, tag="bv_bc")
        ones_bf128 = keep.tile([1, 128], BF16, tag="ones_bf128")

        # ---- pools ------------------------------------------------------
        att = ctx.enter_context(
            tc.tile_pool(name="att", bufs=2, space="PSUM"))      # 2x2 banks
        yps = ctx.enter_context(
            tc.tile_pool(name="yps", bufs=1, space="PSUM"))      # 2x1 bank
        mmp = ctx.enter_context(
            tc.tile_pool(name="mmp", bufs=2, space="PSUM"))      # 2 banks
        norm = ctx.enter_context(tc.tile_pool(name="norm", bufs=3))
        osb = ctx.enter_context(tc.tile_pool(name="osb", bufs=4))

        # ---- input DMAs (SP queue, first-needed first) ------------------
        xT_r = xT_d.ap().rearrange("(o p) t -> p o t", p=128)
        wq_r = wq_d.ap().rearrange("(o p) m -> p o m", p=128)
        wk_r = wk_d.ap().rearrange("(o p) m -> p o m", p=128)
        wv_r = wv_d.ap().rearrange("(o p) m -> p o m", p=128)
        wp_r = wp_d.ap().rearrange("(o p) n -> p o n", p=128)

        def xt_chunk(tb):
            nc.sync.dma_start(
                xT_s[:, :, tb * 512 : (tb + 1) * 512],
                xT_r[:, :, tb * 512 : (tb + 1) * 512],
            )

        nc.sync.dma_start(wq_s[:], wq_r[:])
        nc.sync.dma_start(wk_s[:], wk_r[:])
        xt_chunk(0)
        xt_chunk(1)
        xt_chunk(2)
        xt_chunk(3)
        nc.sync.dma_start(wv_s[:], wv_r[:])
        nc.sync.dma_start(wp_s[:], wp_r[:])
        # consts staged through a DVE copy: consumers then depend on DVE
        # program order instead of a DMA semaphore (walrus 1-wait limit)
        nc.gpsimd.dma_start(tri_st[:], tri_d.ap())
        nc.gpsimd.dma_start(sel_st[:], sel_d.ap())
        nc.gpsimd.dma_start(bq_st[:], bq_d.ap().rearrange("(o p) -> p o", p=128))
        nc.gpsimd.dma_start(bk_st[:], bk_d.ap().rearrange("(o p) -> p o", p=128))
        nc.gpsimd.dma_start(bv_row[:], bv_d.ap()[None, :])
        nc.vector.tensor_copy(tri_s[:], tri_st[:])
        nc.vector.tensor_copy(sel_s[:], sel_st[:])
        nc.vector.tensor_copy(bq_s[:], bq_st[:])
        nc.vector.tensor_copy(bk_s[:], bk_st[:])
        nc.vector.tensor_copy(bv_row_bf[:], bv_row[:])
        nc.vector.memset(ones_bf128[:], 1.0)
        nc.vector.memset(v_s[:, :, :, D : D + 1], 1.0)
        bv_ps = mmp.tile([128, 512], F32, tag="mm", name="bv_ps")
        nc.tensor.matmul(
            bv_ps[:, :MH], ones_bf128[:], bv_row_bf[:], start=True, stop=True
        )
        nc.vector.tensor_copy(bv_bc[:], bv_ps[:, :MH])

        # ---- emit helpers ----------------------------------------------
        def qk_block(w_s, b_s, dst, mt, tb, pool, pool_kw):
            """q/k projection for one (mt, tb): 8 matmuls + DVE drain (the
            ScalarE is the pacing engine during attention, keep it clear)."""
            ps = pool.tile([128, 512], F32, **pool_kw)
            for ct in range(CT):
                nc.tensor.matmul(
                    ps[:],
                    w_s[:, ct, mt * 128 : (mt + 1) * 128],
                    xT_s[:, ct, tb * 512 : (tb + 1) * 512],
                    start=(ct == 0),
                    stop=(ct == CT - 1),
                )
            nc.vector.tensor_scalar(
                dst[:, mt, tb * 512 : (tb + 1) * 512],
                ps[:],
                b_s[:, mt : mt + 1],
                None,
                mybir.AluOpType.add,
            )

        def v_block(tt):
            """v projection for one tt: 8 matmuls + DVE drain (bias add)."""
            ps = mmp.tile([128, 512], F32, tag="mm", name="v_ps")
            for ct in range(CT):
                nc.tensor.matmul(
                    ps[:, :MH],
                    xT_s[:, ct, tt * 128 : (tt + 1) * 128],
                    wv_s[:, ct, :],
                    start=(ct == 0),
                    stop=(ct == CT - 1),
                )
            nc.vector.tensor_tensor(
                v_s[:, tt, :, 0:D],
                ps[:, :MH].rearrange("p (h d) -> p h d", h=LH),
                bv_bc[:].rearrange("p (h d) -> p h d", h=LH),
                mybir.AluOpType.add,
            )

        in_tail = [False]
        ostate = {"n": 0, "ot": None}

        def out_block(tt, nb):
            """out projection for one (tt, nb): 2 matmuls + drain; full
            [128, 1024] DMA per tt (coalesced across nb). PSUM rotates
            mm/sps (att pool in tail); drains alternate DVE/ScalarE;
            DMA queues alternate sync/gpsimd."""
            k = ostate["n"]
            ostate["n"] += 1
            if in_tail[0]:
                ab = att.tile([128, 2, 512], F32, tag="att", name="o_ps")
                ps = ab[:, 0, :]
            else:
                ps = mmp.tile([128, 512], F32, tag="mm", name="o_ps")
            for pp in range(2):
                nc.tensor.matmul(
                    ps,
                    yTn_s[:, pp, tt * 128 : (tt + 1) * 128],
                    wp_s[:, pp, nb * 512 : (nb + 1) * 512],
                    start=(pp == 0),
                    stop=(pp == 1),
                )
            if nb == 0:
                ostate["ot"] = osb.tile(
                    [128, 1024], BF16, tag="out_t", name="out_t"
                )
            ot = ostate["ot"]
            if in_tail[0]:
                nc.scalar.activation(
                    ot[:, nb * 512 : (nb + 1) * 512],
                    ps,
                    mybir.ActivationFunctionType.Copy,
                )
            else:
                nc.vector.tensor_copy(ot[:, nb * 512 : (nb + 1) * 512], ps)
            if nb == 1:
                eng = nc.sync if (tt % 2 == 0) else nc.gpsimd
                eng.dma_start(out_r[tt, :, :], ot[:])

        out_r = out_d.ap().rearrange("(tt p) n -> tt p n", p=128)

        # filler queue: list of zero-arg closures, each one PE quantum
        fillers = []

        def pump(n=1):
            for _ in range(n):
                if fillers:
                    fillers.pop(0)()

        # ---- phase A: q/k projections for pair 0 (att pool as psum) ----
        for tb in range(IB):
            ab = att.tile([128, 2, 512], F32, tag="att", name="ab")
            # use the two halves of an att tile for q and k drains
            for half, (w_s, b_s, dst) in enumerate(
                ((wq_s, bq_s, qT_s), (wk_s, bk_s, kT_s))
            ):
                for ct in range(CT):
                    nc.tensor.matmul(
                        ab[:, half, :],
                        w_s[:, ct, 0:128],
                        xT_s[:, ct, tb * 512 : (tb + 1) * 512],
                        start=(ct == 0),
                        stop=(ct == CT - 1),
                    )
                nc.scalar.activation(
                    dst[:, 0, tb * 512 : (tb + 1) * 512],
                    ab[:, half, :],
                    mybir.ActivationFunctionType.Identity,
                    bias=b_s[:, 0:1],
                )

        # fillers for pair-0 attention phase: q/k mt=1 early (p1 trails
        # need them), v blocks interleaved (yT bursts consume per j-tile)
        for tt in range(TT):
            fillers.append(lambda tt=tt: v_block(tt))
            if tt < IB:
                fillers.append(
                    lambda tb=tt: qk_block(
                        wq_s, bq_s, qT_s, 1, tb, mmp, dict(tag="mm", name="q_ps")
                    )
                )
                fillers.append(
                    lambda tb=tt: qk_block(
                        wk_s, bk_s, kT_s, 1, tb, mmp, dict(tag="mm", name="k_ps")
                    )
                )

        # ---- attention pairs --------------------------------------------
        def attT_trail(p, jt):
            """attT + exp + diag mask for j-tile jt, full causal trail."""
            ia = 128 * jt
            trail = TRAIL[jt]
            off = PT_OFF[jt]
            c = 0
            while c < trail:
                n = min(512, trail - c)
                ab = att.tile([128, 2, 512], F32, tag="att", name="ab")
                for lh in range(2):
                    prow = slice(64 * lh, 64 * lh + 64)
                    nc.tensor.matmul(
                        ab[:, lh, :n],
                        kT_s[prow, p, ia : ia + 128],
                        qT_s[prow, p, ia + c : ia + c + n],
                        start=True,
                        stop=True,
                    )
                pump(1)
                nc.scalar.activation(
                    PT_s[:, :, off + c : off + c + n],
                    ab[:, :, :n],
                    mybir.ActivationFunctionType.Exp,
                    scale=float(SCALE),
                )
                c += n
            # diagonal causal mask on GpSimd (Pool), off the DVE
            for lh in range(2):
                nc.gpsimd.tensor_tensor(
                    PT_s[:, lh, off : off + 128],
                    PT_s[:, lh, off : off + 128],
                    tri_s[:],
                    mybir.AluOpType.mult,
                )

        for p in range(2):
            for ib in range(IB):
                # 4 new j-tiles' attT trails first, then the yT burst
                # (interleaving trails into the burst measured slower:
                # PSUM-group switching breaks PE pipelining)
                for r in range(4):
                    attT_trail(p, 4 * ib + r)
                yts = [
                    yps.tile([D + 1, 512], F32, tag=f"y{lh}", name=f"yt{lh}")
                    for lh in range(2)
                ]
                for jt2 in range(4 * ib + 4):
                    ia2 = 128 * jt2
                    c0 = max(512 * ib, ia2)
                    w = 512 * ib + 512 - c0
                    for lh in range(2):
                        nc.tensor.matmul(
                            yts[lh][:, c0 - 512 * ib : 512],
                            v_s[:, jt2, 2 * p + lh, :],
                            PT_s[:, lh, PT_OFF[jt2] + c0 - ia2 :
                                 PT_OFF[jt2] + c0 - ia2 + w],
                            start=(jt2 == 0),
                            stop=(jt2 == 4 * ib + 3),
                        )
                    pump(1)

                # drain + normalize this ib
                srows = norm.tile([2, 512], F32, tag="srows", name="srows")
                rs2f = norm.tile([2, 512], F32, tag="rs2f", name="rs2f")
                rs2 = norm.tile([2, 512], BF16, tag="rs2", name="rs2")
                yTall = [
                    norm.tile([D + 1, 512], F32, tag=f"yTall{lh}", name=f"yTall{lh}")
                    for lh in range(2)
                ]
                for lh in range(2):
                    nc.scalar.activation(
                        yTall[lh][:], yts[lh][:],
                        mybir.ActivationFunctionType.Copy,
                    )
                    nc.sync.dma_start(srows[lh : lh + 1, :], yTall[lh][D : D + 1, :])
                nc.vector.reciprocal_approx_fast(rs2f[:], srows[:])
                nc.vector.tensor_copy(rs2[:], rs2f[:])
                pump(1)
                S_ps = mmp.tile([128, 512], F32, tag="mm", name="S_ps")
                nc.tensor.matmul(S_ps[:], sel_s[:], rs2[:], start=True, stop=True)
                for lh in range(2):
                    nc.vector.tensor_tensor(
                        yTn_s[64 * lh : 64 * lh + 64, p, 512 * ib : 512 * ib + 512],
                        yTall[lh][0:D, :],
                        S_ps[64 * lh : 64 * lh + 64, :],
                        mybir.AluOpType.mult,
                    )
                if p == 1:
                    # both pairs normalized for this ib: queue out-proj fillers
                    for tt in range(4 * ib, 4 * ib + 4):
                        for nb in range(2):
                            fillers.append(
                                lambda tt=tt, nb=nb: out_block(tt, nb)
                            )

        # tail: drain remaining fillers (last ib's out-proj) on att pool
        in_tail[0] = True
        while fillers:
            f = fillers.pop(0)
            f()


_NC_CACHE = None


def get_nc() -> bass.Bass:
    global _NC_CACHE
    if _NC_CACHE is None:
        nc = bacc.Bacc()
        xT_d = nc.declare_dram_parameter("xT", [C, T], BF16, isOutput=False)
        wq_d = nc.declare_dram_parameter("wq", [C, MH], BF16, isOutput=False)
        wk_d = nc.declare_dram_parameter("wk", [C, MH], BF16, isOutput=False)
        wv_d = nc.declare_dram_parameter("wv", [C, MH], BF16, isOutput=False)
        wp_d = nc.declare_dram_parameter("wp", [MH, C], BF16, isOutput=False)
        bq_d = nc.declare_dram_parameter("bq", [MH], F32, isOutput=False)
        bk_d = nc.declare_dram_parameter("bk", [MH], F32, isOutput=False)
        bv_d = nc.declare_dram_parameter("bv", [MH], F32, isOutput=False)
        out_d = nc.declare_dram_parameter("out", [T, C], BF16, isOutput=True)
        tri_d = nc.inline_tensor(_tri_mask(), name="tri_mask")
        sel_d = nc.inline_tensor(_selector(), name="selector")
        emit_kernel(
            nc, xT_d, wq_d, wk_d, wv_d, wp_d, bq_d, bk_d, bv_d, out_d, tri_d, sel_d
        )
        nc.finalize()
        _NC_CACHE = nc
    return _NC_CACHE


def make_in_maps(x, Wq, bq, Wk, bk, Wv, bv, Wp, bp):
    in_maps = []
    for core in range(N_CORES):
        b, g = divmod(core, GROUPS)
        sl = slice(g * MH, (g + 1) * MH)
        in_maps.append(
            {
                "xT": np.ascontiguousarray(x[b].T).astype(NP_BF16),
                "wq": np.ascontiguousarray(Wq[:, sl]).astype(NP_BF16),
                "wk": np.ascontiguousarray(Wk[:, sl]).astype(NP_BF16),
                "wv": np.ascontiguousarray(Wv[:, sl]).astype(NP_BF16),
                "wp": np.ascontiguousarray(Wp[sl, :]).astype(NP_BF16),
                "bq": np.ascontiguousarray(bq[sl]).astype(np.float32),
                "bk": np.ascontiguousarray(bk[sl]).astype(np.float32),
                "bv": np.ascontiguousarray(bv[sl]).astype(np.float32),
            }
        )
    return in_maps


def kernel(x, Wq, bq, Wk, bk, Wv, bv, Wp, bp, _results_hook=None, _trace=False):
    x = np.asarray(x, dtype=np.float32)
    nc = get_nc()
    in_maps = make_in_maps(x, Wq, bq, Wk, bk, Wv, bv, Wp, bp)
    res = run_bass_kernel_spmd(
        nc, in_maps, core_ids=list(range(N_CORES)), trace=_trace
    )
    if _results_hook is not None:
        _results_hook(res)
    out = np.zeros((B, T, C), dtype=np.float32)
    for core in range(N_CORES):
        b = core // GROUPS
        out[b] += res.results[core]["out"].astype(np.float32)
    out += np.asarray(bp, dtype=np.float32)[None, None, :]
    return out
